# revision 1
# baseline (speedup 1.0000x reference)
"""GCNII (nn_GCNII_17626545783193) Bass/Trainium2 kernel, 8 NeuronCores.

Strategy (target-node sharding, feature-major compute):
  - Nodes sharded 12500/core (padded to 12544 = 98*128). Edges partitioned
    by target core; self-loops included as real edges.
  - gcn_norm factored: norm[e] = dinv[src]*dinv[dst]. The dinv[src] factor is
    folded into the gathered features (hs = dinv * h, replicated via AllGather
    each layer); 0.9*dinv[dst] is folded into the one-hot scatter matrix.
  - Per layer, per core: dma_gather of in-edge source rows (256B each) from
    hs_full; DVE builds one-hot [128edge, 128target] tiles (iota==tloc)*dcol;
    PE computes agg^T = msgs^T @ onehot accumulated in PSUM [64, 512];
    support = agg + 0.1*h0^T; GCNII dense update via two accumulating
    matmuls (beta*Wl and (1-beta)*I, host-prescaled); ACT relu; PE transpose
    back to node-major; scale by dinv; AllGather.
"""
import sys
sys.path.insert(0, "/opt/trn_rl_repo")

import math
import numpy as np

# ---- problem constants (hardcoded per spec) ----
N_NODES = 100000
N_FEATS = 128
HIDDEN = 64
N_CLASSES = 40
N_LAYERS = 8
ALPHA = 0.1
LAMDA = 0.5
NCORES = 8

NPC = N_NODES // NCORES            # 12500 real nodes per core
TPC = (NPC + 127) // 128           # 98 target tiles per core
NPC_PAD = TPC * 128                # 12544
N_PAD = NPC_PAD * NCORES           # 100352
GROUP_TT = 8                       # target tiles per gather group
SUB_TT = 4                         # target tiles per psum tile (512 cols)
SRC_CHUNK = 32768                  # int16 index range per gather source chunk
NCHUNKS = (N_PAD + SRC_CHUNK - 1) // SRC_CHUNK  # 4
CALL_MAX = 8192                    # max indices per dma_gather call

TRACE = False          # test.py sets this for profiling
_LAST_RESULT = {}      # test.py reads exec_time from here


def _preprocess(edge_index):
    """Build the uniform (cross-core identical) edge stream structure and the
    per-core data arrays."""
    row = np.asarray(edge_index[0], dtype=np.int64)
    col = np.asarray(edge_index[1], dtype=np.int64)
    deg = np.bincount(col, minlength=N_NODES).astype(np.float32) + 1.0
    dinv = (1.0 / np.sqrt(deg)).astype(np.float32)

    loops = np.arange(N_NODES, dtype=np.int64)
    r = np.concatenate([row, loops])
    cl = np.concatenate([col, loops])
    core = cl // NPC
    lcol = cl - core * NPC                       # 0..12499 local target
    tt = lcol >> 7                               # target tile 0..97
    gp = (r // NPC) * NPC_PAD + (r % NPC)        # padded global source id
    k = gp // SRC_CHUNK                          # source chunk 0..3

    ngroups = (TPC + GROUP_TT - 1) // GROUP_TT

    slot = (core * TPC + tt) * NCHUNKS + k
    counts = np.bincount(slot, minlength=NCORES * TPC * NCHUNKS).reshape(
        NCORES, TPC, NCHUNKS
    )
    maxc = counts.max(axis=0)                    # [TPC, NCHUNKS]
    slot_chunks = -(-maxc // 128)                # ceil; 0 allowed
    slot_len = slot_chunks * 128

    slot_off = np.zeros((TPC, NCHUNKS), np.int64)
    off = 0
    chunk_tt = []
    calls_by_group = [[] for _ in range(ngroups)]
    for g in range(ngroups):
        tts = range(g * GROUP_TT, min((g + 1) * GROUP_TT, TPC))
        for kk in range(NCHUNKS):
            run_start = off
            for t in tts:
                slot_off[t, kk] = off
                off += slot_len[t, kk]
                chunk_tt += [t] * slot_chunks[t, kk]
            s = run_start
            while s < off:
                n = min(CALL_MAX, off - s)
                calls_by_group[g].append((kk, int(s), int(n)))
                s += n
    e_pad = off
    nchk = e_pad // 128
    chunk_tt = np.asarray(chunk_tt, np.int64)
    # start/stop are per PSUM tile (= subgroup of SUB_TT ttiles): exactly one
    # start=True (chronologically first matmul into the bank) and one
    # stop=True (last). A second start into a live bank wipes it (bank-wide
    # pending-zero), so interleaved per-ttile groups are NOT allowed.
    sub_id = chunk_tt // SUB_TT
    first = np.zeros(nchk, bool)
    last = np.zeros(nchk, bool)
    for sg in np.unique(sub_id):
        js = np.nonzero(sub_id == sg)[0]
        first[js[0]] = True
        last[js[-1]] = True

    per_core = []
    for c in range(NCORES):
        m = core == c
        rc = r[m]
        clm = cl[m]
        lc = lcol[m]
        tc_ = tt[m]
        kc = k[m]
        gpc = gp[m]
        skey = tc_ * NCHUNKS + kc
        order = np.argsort(skey, kind="stable")
        sorted_key = skey[order]
        firsts = np.searchsorted(sorted_key, sorted_key, side="left")
        ranks = np.arange(len(sorted_key)) - firsts
        pos = slot_off[tc_[order], kc[order]] + ranks

        idx_stream = np.zeros(e_pad, np.int16)
        tloc_stream = np.zeros(e_pad, np.float32)
        dcol_stream = np.zeros(e_pad, np.float32)
        idx_stream[pos] = (gpc[order] - kc[order] * SRC_CHUNK).astype(np.int16)
        tloc_stream[pos] = (lc[order] & 127).astype(np.float32)
        dcol_stream[pos] = ((1.0 - ALPHA) * dinv[clm[order]]).astype(np.float32)

        idx_w = np.zeros((128, e_pad // 16), np.int16)
        for g in range(ngroups):
            for (kk, s, n) in calls_by_group[g]:
                blk = idx_stream[s:s + n].reshape(n // 16, 16).T
                idx_w[:, s // 16:(s + n) // 16] = np.tile(blk, (8, 1))
        tloc_arr = np.ascontiguousarray(tloc_stream.reshape(nchk, 128).T)
        dcol_arr = np.ascontiguousarray(dcol_stream.reshape(nchk, 128).T)
        dv = np.zeros(NPC_PAD, np.float32)
        dv[:NPC] = dinv[c * NPC:(c + 1) * NPC]
        dinvo = np.ascontiguousarray(dv.reshape(TPC, 128).T)
        per_core.append(
            dict(idx=idx_w, tloc=tloc_arr, dcol=dcol_arr, dinvo=dinvo)
        )

    struct = dict(
        e_pad=e_pad,
        nchk=nchk,
        ngroups=ngroups,
        calls_by_group=calls_by_group,
        chunk_tt=chunk_tt,
        first=first,
        last=last,
    )
    return struct, per_core


def _build_program(struct):
    import concourse.bass as bass
    import concourse.mybir as mybir
    import concourse.tile as tile
    import concourse.bacc as bacc

    dt = mybir.dt
    f32 = dt.float32
    Alu = mybir.AluOpType
    Act = mybir.ActivationFunctionType

    e_pad = struct["e_pad"]
    nchk = struct["nchk"]
    ngroups = struct["ngroups"]
    calls_by_group = struct["calls_by_group"]
    chunk_tt = struct["chunk_tt"]
    first = struct["first"]
    last = struct["last"]
    rg = [list(range(NCORES))]

    nc = bacc.Bacc("TRN2", target_bir_lowering=False, debug=False,
                   num_devices=NCORES)

    x_in = nc.dram_tensor("x", [NPC_PAD, N_FEATS], f32, kind="ExternalInput")
    idx_in = nc.dram_tensor("idx", [128, e_pad // 16], dt.int16,
                            kind="ExternalInput")
    tloc_in = nc.dram_tensor("tloc", [128, nchk], f32, kind="ExternalInput")
    dcol_in = nc.dram_tensor("dcol", [128, nchk], f32, kind="ExternalInput")
    dinvo_in = nc.dram_tensor("dinvo", [128, TPC], f32, kind="ExternalInput")
    w0_in = nc.dram_tensor("w0", [N_FEATS, HIDDEN], f32, kind="ExternalInput")
    b0_in = nc.dram_tensor("b0", [HIDDEN, 1], f32, kind="ExternalInput")
    wl_in = nc.dram_tensor("wl", [HIDDEN, N_LAYERS * HIDDEN], f32,
                           kind="ExternalInput")
    il_in = nc.dram_tensor("il", [HIDDEN, N_LAYERS * HIDDEN], f32,
                           kind="ExternalInput")
    wout_in = nc.dram_tensor("wout", [HIDDEN, N_CLASSES], f32,
                             kind="ExternalInput")
    bout_in = nc.dram_tensor("bout", [N_CLASSES, 1], f32, kind="ExternalInput")
    iota_in = nc.dram_tensor("iota", [128, 128], f32, kind="ExternalInput")
    ident_in = nc.dram_tensor("ident", [128, 128], f32, kind="ExternalInput")
    out_dram = nc.dram_tensor("out", [NPC_PAD, N_CLASSES], f32,
                              kind="ExternalOutput")

    with tile.TileContext(nc) as tc:
        with (
            tc.tile_pool(name="persist", bufs=1) as P,
            tc.tile_pool(name="work", bufs=3) as S,
            tc.tile_pool(name="msgp", bufs=3) as MSG,
            tc.tile_pool(name="ohp", bufs=6) as OH,
            tc.tile_pool(name="psagg", bufs=4, space="PSUM") as PSA,
            tc.tile_pool(name="psmisc", bufs=2, space="PSUM") as PSM,
            tc.tile_pool(name="dram", bufs=1, space="DRAM") as DR,
        ):
            def pload(name, src, shape, dtype=f32):
                t = P.tile(shape, dtype, tag=name)
                nc.sync.dma_start(t[:], src[:])
                return t

            iota = pload("iota", iota_in, [128, 128])
            ident = pload("ident", ident_in, [128, 128])
            w0 = pload("w0", w0_in, [N_FEATS, HIDDEN])
            b0 = pload("b0", b0_in, [HIDDEN, 1])
            wl = pload("wl", wl_in, [HIDDEN, N_LAYERS * HIDDEN])
            il = pload("il", il_in, [HIDDEN, N_LAYERS * HIDDEN])
            wout = pload("wout", wout_in, [HIDDEN, N_CLASSES])
            bout = pload("bout", bout_in, [N_CLASSES, 1])
            tloc = pload("tloc", tloc_in, [128, nchk])
            dcol = pload("dcol", dcol_in, [128, nchk])
            dinvo = pload("dinvo", dinvo_in, [128, TPC])
            h0s = P.tile([HIDDEN, NPC_PAD], f32, tag="h0s")

            hs_shard = [
                DR.tile([NPC_PAD, HIDDEN], f32, tag=f"shard{j}",
                        name=f"hs_shard{j}")
                for j in range(N_LAYERS)
            ]
            hs_full = [
                DR.tile([N_PAD, HIDDEN], f32, tag=f"full{j}",
                        addr_space="Shared", name=f"hs_full{j}")
                for j in range(N_LAYERS)
            ]

            # ---------------- layer 0: h0 = relu(x @ W0 + b0) ----------------
            for t in range(TPC):
                xt = S.tile([128, N_FEATS], f32, tag="xt")
                nc.sync.dma_start(xt[:], x_in[t * 128:(t + 1) * 128, :])
                xT_ps = PSM.tile([N_FEATS, 128], f32, tag="tp")
                nc.tensor.transpose(xT_ps[:], xt[:], ident[:])
                xT = S.tile([N_FEATS, 128], f32, tag="xT")
                nc.vector.tensor_copy(xT[:], xT_ps[:])
                h_ps = PSM.tile([HIDDEN, 128], f32, tag="dense")
                nc.tensor.matmul(h_ps[:], lhsT=w0[:], rhs=xT[:],
                                 start=True, stop=True)
                h0t = S.tile([HIDDEN, 128], f32, tag="h0t")
                nc.scalar.activation(h0t[:], h_ps[:], Act.Relu, bias=b0[:])
                nc.vector.tensor_scalar_mul(
                    h0s[:, t * 128:(t + 1) * 128], h0t[:], ALPHA)
                tp_ps = PSM.tile([128, HIDDEN], f32, tag="tp")
                nc.tensor.transpose(tp_ps[:], h0t[:],
                                    ident[:HIDDEN, :HIDDEN])
                hs_sb = S.tile([128, HIDDEN], f32, tag="hs")
                nc.vector.tensor_scalar(hs_sb[:], tp_ps[:],
                                        dinvo[:, t:t + 1], None,
                                        op0=Alu.mult)
                nc.sync.dma_start(
                    hs_shard[0][t * 128:(t + 1) * 128, :], hs_sb[:])
            nc.gpsimd.collective_compute(
                "AllGather", Alu.bypass, replica_groups=rg,
                ins=[hs_shard[0].opt()], outs=[hs_full[0].opt()])

            # ---------------- GCNII layers ----------------
            for li in range(N_LAYERS):
                cur = hs_full[li]
                is_last = li == N_LAYERS - 1
                nxt_shard = hs_shard[li + 1] if not is_last else None
                for g in range(ngroups):
                    tts = list(range(g * GROUP_TT,
                                     min((g + 1) * GROUP_TT, TPC)))
                    subs = [tts[i:i + SUB_TT]
                            for i in range(0, len(tts), SUB_TT)]
                    ps_tiles = [PSA.tile([HIDDEN, SUB_TT * 128], f32,
                                         tag="agg", name=f"agg{li}_{g}_{si}")
                                for si in range(len(subs))]
                    for (kk, s, n) in calls_by_group[g]:
                        idx_t = S.tile([128, n // 16], dt.int16, tag="idxs")
                        nc.sync.dma_start(
                            idx_t[:], idx_in[:, s // 16:(s + n) // 16])
                        msg = MSG.tile([128, n // 128, HIDDEN], f32,
                                       tag="msg")
                        rows_k = min(SRC_CHUNK, N_PAD - kk * SRC_CHUNK)
                        nc.gpsimd.dma_gather(
                            msg[:],
                            cur[kk * SRC_CHUNK:kk * SRC_CHUNK + rows_k, :],
                            idx_t[:], num_idxs=n, num_idxs_reg=n,
                            elem_size=HIDDEN, single_packet=False)
                        for jj in range(n // 128):
                            j = s // 128 + jj
                            t = int(chunk_tt[j])
                            oh = OH.tile([128, 128], f32, tag="oh")
                            nc.vector.tensor_scalar(
                                oh[:], iota[:], tloc[:, j:j + 1],
                                dcol[:, j:j + 1],
                                op0=Alu.is_equal, op1=Alu.mult)
                            si = (t - g * GROUP_TT) // SUB_TT
                            ci = (t % SUB_TT) * 128
                            nc.tensor.matmul(
                                ps_tiles[si][:, ci:ci + 128],
                                lhsT=msg[:, jj, :], rhs=oh[:],
                                start=bool(first[j]), stop=bool(last[j]),
                                skip_group_check=True)
                    for si, stts in enumerate(subs):
                        w = len(stts) * 128
                        n0 = stts[0] * 128
                        sup = S.tile([HIDDEN, SUB_TT * 128], f32, tag="sup")
                        nc.vector.tensor_tensor(
                            out=sup[:, :w], in0=ps_tiles[si][:, :w],
                            in1=h0s[:, n0:n0 + w], op=Alu.add)
                        d_ps = PSM.tile([HIDDEN, SUB_TT * 128], f32,
                                        tag="dense")
                        nc.tensor.matmul(
                            d_ps[:, :w],
                            lhsT=wl[:, li * HIDDEN:(li + 1) * HIDDEN],
                            rhs=sup[:, :w], start=True, stop=False)
                        nc.tensor.matmul(
                            d_ps[:, :w],
                            lhsT=il[:, li * HIDDEN:(li + 1) * HIDDEN],
                            rhs=sup[:, :w], start=False, stop=True)
                        ht = S.tile([HIDDEN, SUB_TT * 128], f32, tag="ht")
                        nc.scalar.activation(ht[:, :w], d_ps[:, :w], Act.Relu)
                        if not is_last:
                            for tti, t in enumerate(stts):
                                tp_ps = PSM.tile([128, HIDDEN], f32,
                                                 tag="tp")
                                nc.tensor.transpose(
                                    tp_ps[:],
                                    ht[:, tti * 128:(tti + 1) * 128],
                                    ident[:HIDDEN, :HIDDEN])
                                hs_sb = S.tile([128, HIDDEN], f32, tag="hs")
                                nc.vector.tensor_scalar(
                                    hs_sb[:], tp_ps[:], dinvo[:, t:t + 1],
                                    None, op0=Alu.mult)
                                nc.sync.dma_start(
                                    nxt_shard[t * 128:(t + 1) * 128, :],
                                    hs_sb[:])
                        else:
                            o_ps = PSM.tile([N_CLASSES, SUB_TT * 128], f32,
                                            tag="dense")
                            nc.tensor.matmul(o_ps[:, :w], lhsT=wout[:],
                                             rhs=ht[:, :w],
                                             start=True, stop=True)
                            o_sb = S.tile([N_CLASSES, SUB_TT * 128], f32,
                                          tag="osb")
                            nc.vector.tensor_scalar(
                                o_sb[:, :w], o_ps[:, :w], bout[:], None,
                                op0=Alu.add)
                            for tti, t in enumerate(stts):
                                tp_ps = PSM.tile([128, HIDDEN], f32,
                                                 tag="tp")
                                nc.tensor.transpose(
                                    tp_ps[:, :N_CLASSES],
                                    o_sb[:, tti * 128:(tti + 1) * 128],
                                    ident[:N_CLASSES, :N_CLASSES])
                                ot = S.tile([128, N_CLASSES], f32, tag="ot")
                                nc.vector.tensor_copy(
                                    ot[:], tp_ps[:, :N_CLASSES])
                                nc.sync.dma_start(
                                    out_dram[t * 128:(t + 1) * 128, :],
                                    ot[:])
                if not is_last:
                    nc.gpsimd.collective_compute(
                        "AllGather", Alu.bypass, replica_groups=rg,
                        ins=[nxt_shard.opt()],
                        outs=[hs_full[li + 1].opt()])

    nc.compile()
    return nc


def kernel(x, edge_index, W0, b0, Wl, W_out, b_out):
    from concourse.bass_utils import run_bass_kernel_spmd

    x = np.asarray(x, dtype=np.float32)
    edge_index = np.asarray(edge_index)
    W0 = np.asarray(W0, dtype=np.float32)
    b0 = np.asarray(b0, dtype=np.float32)
    Wl = np.asarray(Wl, dtype=np.float32)
    W_out = np.asarray(W_out, dtype=np.float32)
    b_out = np.asarray(b_out, dtype=np.float32)

    struct, per_core = _preprocess(edge_index)
    nc = _build_program(struct)

    betas = np.array(
        [math.log(LAMDA / (i + 1) + 1.0) for i in range(N_LAYERS)],
        dtype=np.float32)
    wl_host = np.zeros((HIDDEN, N_LAYERS * HIDDEN), np.float32)
    il_host = np.zeros((HIDDEN, N_LAYERS * HIDDEN), np.float32)
    eye = np.eye(HIDDEN, dtype=np.float32)
    for i in range(N_LAYERS):
        wl_host[:, i * HIDDEN:(i + 1) * HIDDEN] = betas[i] * Wl[i]
        il_host[:, i * HIDDEN:(i + 1) * HIDDEN] = (1.0 - betas[i]) * eye
    iota_host = np.tile(np.arange(128, dtype=np.float32), (128, 1))
    ident_host = np.eye(128, dtype=np.float32)

    in_maps = []
    for c in range(NCORES):
        xc = np.zeros((NPC_PAD, N_FEATS), np.float32)
        xc[:NPC] = x[c * NPC:(c + 1) * NPC]
        pc = per_core[c]
        in_maps.append({
            "x": xc,
            "idx": pc["idx"],
            "tloc": pc["tloc"],
            "dcol": pc["dcol"],
            "dinvo": pc["dinvo"],
            "w0": W0,
            "b0": b0.reshape(HIDDEN, 1),
            "wl": wl_host,
            "il": il_host,
            "wout": W_out,
            "bout": b_out.reshape(N_CLASSES, 1),
            "iota": iota_host,
            "ident": ident_host,
        })

    res = run_bass_kernel_spmd(
        nc, in_maps, core_ids=list(range(NCORES)), trace=TRACE)
    _LAST_RESULT["res"] = res
    out = np.concatenate(
        [res.results[c]["out"][:NPC] for c in range(NCORES)], axis=0)
    return out



# revision 12
# speedup vs baseline: 1.4323x; 1.4323x over previous
"""GCNII (nn_GCNII_17626545783193) Bass/Trainium2 kernel, 8 NeuronCores.

Strategy (target-node sharding, feature-major compute, fp16 data path):
  - Nodes sharded 12500/core (padded to 12544 = 98*128). Edges partitioned
    by target core; self-loops handled via per-tile diagonal matmuls against
    a locally kept node-major hs copy (not in the gather stream).
  - Per-core target permutation balances per-(ttile, chunk) edge counts
    across tiles AND cores, minimizing the 128-slot padding of the uniform
    (cross-core identical) edge stream.
  - gcn_norm factored: norm[e] = dinv[src]*dinv[dst]. dinv[src] is folded
    into the gathered features (hs = dinv * h, fp16 [N, 128]-padded rows,
    256B, replicated via Shared-output AllGather each layer); 0.9*dinv[dst]
    is folded into the one-hot scatter matrix.
  - Per layer, per core: dma_gather of in-edge source rows (256B fp16) from
    hs_full; DVE builds one-hot [128edge, 128target] fp16 (iota==tloc)*dcol;
    PE computes agg^T = msgs^T @ onehot accumulated in PSUM [64, 512] fp32;
    support = agg + 0.1*h0 (fp32 h0, fp16 out); GCNII dense update via ONE
    matmul with host-merged W' = beta*Wl + (1-beta)*I; ACT relu; PE transpose
    to node-major; scale by dinv; strided write into the padded hs rows;
    AllGather.
"""
import sys
sys.path.insert(0, "/opt/trn_rl_repo")

import math
import numpy as np

# ---- problem constants (hardcoded per spec) ----
N_NODES = 100000
N_FEATS = 128
HIDDEN = 64
N_CLASSES = 40
N_LAYERS = 8
ALPHA = 0.1
LAMDA = 0.5
NCORES = 8

NPC = N_NODES // NCORES            # 12500 real nodes per core
TPC = (NPC + 127) // 128           # 98 target tiles per core
NPC_PAD = TPC * 128                # 12544
N_PAD = NPC_PAD * NCORES           # 100352
HROW = 128                         # padded fp16 row elems (256B)
GROUP_TT = 8                       # target tiles per gather group
SUB_TT = 4                         # target tiles per psum tile (512 cols)
SRC_CHUNK = 32768                  # int16 index range per gather source chunk
NCHUNKS = (N_PAD + SRC_CHUNK - 1) // SRC_CHUNK  # 4
CALL_MAX = 8192                    # max indices per dma_gather call

TRACE = False          # test.py sets this for profiling
_LAST_RESULT = {}      # test.py reads exec_time from here


def _balance_tiles(deg_ck):
    """Assign 12500 local targets to 98 tiles of <=128, balancing total
    degree: sort by degree desc, snake-deal. Returns perm: perm[new_pos] =
    old local id, laid out tile-major (tile t = perm[t*128:(t+1)*128],
    padded with -1)."""
    tot = deg_ck.sum(axis=1)
    order = np.argsort(-tot, kind="stable")
    tiles = [[] for _ in range(TPC)]
    tsum = np.zeros(TPC)
    # snake deal in rounds of TPC
    i = 0
    fwd = True
    while i < len(order):
        rng = range(TPC) if fwd else range(TPC - 1, -1, -1)
        for t in rng:
            if i >= len(order):
                break
            if len(tiles[t]) < 128:
                tiles[t].append(order[i])
                tsum[t] += tot[order[i]]
                i += 1
        fwd = not fwd
    # order tiles by total degree desc so maxima align across cores
    tile_order = np.argsort(-tsum, kind="stable")
    perm = np.full(NPC_PAD, -1, np.int64)
    for newt, oldt in enumerate(tile_order):
        ids = tiles[oldt]
        perm[newt * 128:newt * 128 + len(ids)] = ids
    return perm


def _preprocess(edge_index):
    """Build the uniform (cross-core identical) edge stream structure and the
    per-core data arrays (with per-core balancing permutations)."""
    row = np.asarray(edge_index[0], dtype=np.int64)
    col = np.asarray(edge_index[1], dtype=np.int64)
    deg = np.bincount(col, minlength=N_NODES).astype(np.float32) + 1.0
    dinv = (1.0 / np.sqrt(deg)).astype(np.float32)

    # self-loops are handled on-chip via a per-tile diagonal matmul against
    # the locally-kept node-major hs copy — they are NOT in the edge stream.
    r = row
    cl = col
    core = cl // NPC
    lcol = cl - core * NPC                       # 0..12499 local target

    # ---- per-core balancing permutation over local targets ----
    src_core = r // NPC
    src_local = r - src_core * NPC
    perms = []            # perm[new_pos (padded)] = old local id
    inv_perms = []        # inv[old local id] = new_pos (padded)
    new_lcol = np.empty_like(lcol)
    for c in range(NCORES):
        m = core == c
        # per-target degree per source chunk (chunk of the *old* padded gp;
        # balancing on totals is enough, chunks are ~proportional)
        deg_c = np.bincount(lcol[m], minlength=NPC)
        perm = _balance_tiles(deg_c.reshape(-1, 1).astype(np.float64))
        inv = np.full(NPC, -1, np.int64)
        valid = perm >= 0
        inv[perm[valid]] = np.nonzero(valid)[0]
        perms.append(perm)
        inv_perms.append(inv)
        new_lcol[m] = inv[lcol[m]]
    lcol = new_lcol                               # padded-permuted local target

    # padded global source id (through the source core's permutation)
    gp = np.empty_like(r)
    for c in range(NCORES):
        m = src_core == c
        gp[m] = c * NPC_PAD + inv_perms[c][src_local[m]]

    tt = lcol >> 7                               # target tile 0..97
    k = gp // SRC_CHUNK                          # source chunk 0..3

    ngroups = (TPC + GROUP_TT - 1) // GROUP_TT

    slot = (core * TPC + tt) * NCHUNKS + k
    counts = np.bincount(slot, minlength=NCORES * TPC * NCHUNKS).reshape(
        NCORES, TPC, NCHUNKS
    )
    maxc = counts.max(axis=0)                    # [TPC, NCHUNKS]
    slot_chunks = -(-maxc // 128)                # ceil; 0 allowed
    slot_len = slot_chunks * 128

    slot_off = np.zeros((TPC, NCHUNKS), np.int64)
    off = 0
    chunk_tt = []
    calls_by_group = [[] for _ in range(ngroups)]
    for g in range(ngroups):
        tts = range(g * GROUP_TT, min((g + 1) * GROUP_TT, TPC))
        for kk in range(NCHUNKS):
            run_start = off
            for t in tts:
                slot_off[t, kk] = off
                off += slot_len[t, kk]
                chunk_tt += [t] * slot_chunks[t, kk]
            s = run_start
            while s < off:
                n = min(CALL_MAX, off - s)
                calls_by_group[g].append((kk, int(s), int(n)))
                s += n
    e_pad = off
    nchk = e_pad // 128
    chunk_tt = np.asarray(chunk_tt, np.int64)
    # start/stop are per PSUM tile (= subgroup of SUB_TT ttiles): exactly one
    # start=True (chronologically first matmul into the bank) and one
    # stop=True (last). A second start into a live bank wipes it (bank-wide
    # pending-zero), so interleaved per-ttile groups are NOT allowed.
    sub_id = chunk_tt // SUB_TT
    first = np.zeros(nchk, bool)
    last = np.zeros(nchk, bool)
    for sg in np.unique(sub_id):
        js = np.nonzero(sub_id == sg)[0]
        first[js[0]] = True
        last[js[-1]] = True

    per_core = []
    for c in range(NCORES):
        m = core == c
        rc = r[m]
        clm = cl[m]
        lc = lcol[m]
        tc_ = tt[m]
        kc = k[m]
        gpc = gp[m]
        skey = tc_ * NCHUNKS + kc
        order = np.argsort(skey, kind="stable")
        sorted_key = skey[order]
        firsts = np.searchsorted(sorted_key, sorted_key, side="left")
        ranks = np.arange(len(sorted_key)) - firsts
        pos = slot_off[tc_[order], kc[order]] + ranks

        idx_stream = np.zeros(e_pad, np.int16)
        tloc_stream = np.zeros(e_pad, np.float32)
        dcol_stream = np.zeros(e_pad, np.float32)
        idx_stream[pos] = (gpc[order] - kc[order] * SRC_CHUNK).astype(np.int16)
        tloc_stream[pos] = (lc[order] & 127).astype(np.float32)
        dcol_stream[pos] = ((1.0 - ALPHA) * dinv[clm[order]]).astype(np.float32)

        idx_w = np.zeros((128, e_pad // 16), np.int16)
        for g in range(ngroups):
            for (kk, s, n) in calls_by_group[g]:
                blk = idx_stream[s:s + n].reshape(n // 16, 16).T
                idx_w[:, s // 16:(s + n) // 16] = np.tile(blk, (8, 1))
        tloc_arr = np.ascontiguousarray(tloc_stream.reshape(nchk, 128).T)
        dcol_arr = np.ascontiguousarray(dcol_stream.reshape(nchk, 128).T)
        dv = np.zeros(NPC_PAD, np.float32)
        pm = perms[c]
        valid = pm >= 0
        dv[valid] = dinv[c * NPC + pm[valid]]
        dinvo = np.ascontiguousarray(dv.reshape(TPC, 128).T)
        per_core.append(
            dict(idx=idx_w, tloc=tloc_arr, dcol=dcol_arr, dinvo=dinvo,
                 perm=pm)
        )

    struct = dict(
        e_pad=e_pad,
        nchk=nchk,
        ngroups=ngroups,
        calls_by_group=calls_by_group,
        chunk_tt=chunk_tt,
        first=first,
        last=last,
    )
    return struct, per_core


def _build_program(struct):
    import concourse.bass as bass
    import concourse.mybir as mybir
    import concourse.tile as tile
    import concourse.bacc as bacc

    dt = mybir.dt
    f32 = dt.float32
    f16 = dt.float16
    Alu = mybir.AluOpType
    Act = mybir.ActivationFunctionType

    e_pad = struct["e_pad"]
    nchk = struct["nchk"]
    ngroups = struct["ngroups"]
    calls_by_group = struct["calls_by_group"]
    chunk_tt = struct["chunk_tt"]
    first = struct["first"]
    last = struct["last"]
    rg = [list(range(NCORES))]

    nc = bacc.Bacc("TRN2", target_bir_lowering=False, debug=False,
                   num_devices=NCORES)

    xt_in = nc.dram_tensor("xt", [N_FEATS, NPC_PAD], f16, kind="ExternalInput")
    idx_in = nc.dram_tensor("idx", [128, e_pad // 16], dt.int16,
                            kind="ExternalInput")
    tloc_in = nc.dram_tensor("tloc", [128, nchk], f32, kind="ExternalInput")
    dcol_in = nc.dram_tensor("dcol", [128, nchk], f32, kind="ExternalInput")
    dinvo_in = nc.dram_tensor("dinvo", [128, TPC], f32, kind="ExternalInput")
    w0_in = nc.dram_tensor("w0", [N_FEATS, HIDDEN], f16, kind="ExternalInput")
    b0_in = nc.dram_tensor("b0", [HIDDEN, 1], f32, kind="ExternalInput")
    wl_in = nc.dram_tensor("wl", [HIDDEN, N_LAYERS * HIDDEN], f16,
                           kind="ExternalInput")
    wout_in = nc.dram_tensor("wout", [HIDDEN, N_CLASSES], f16,
                             kind="ExternalInput")
    bout_in = nc.dram_tensor("bout", [N_CLASSES, 1], f32, kind="ExternalInput")
    iota_in = nc.dram_tensor("iota", [128, 128], f16, kind="ExternalInput")
    ident_in = nc.dram_tensor("ident", [128, 128], f16, kind="ExternalInput")
    out_dram = nc.dram_tensor("out", [NPC_PAD, N_CLASSES], f32,
                              kind="ExternalOutput")

    with tile.TileContext(nc) as tc:
        with (
            tc.tile_pool(name="persist", bufs=1) as P,
            tc.tile_pool(name="work", bufs=3) as S,
            tc.tile_pool(name="msgp", bufs=3) as MSG,
            tc.tile_pool(name="ohp", bufs=8) as OH,
            tc.tile_pool(name="psagg", bufs=4, space="PSUM") as PSA,
            tc.tile_pool(name="psmisc", bufs=2, space="PSUM") as PSM,
            tc.tile_pool(name="dram", bufs=1, space="DRAM") as DR,
        ):
            def pload(name, src, shape, dtype=f32):
                t = P.tile(shape, dtype, tag=name)
                nc.sync.dma_start(t[:], src[:])
                return t

            iota = pload("iota", iota_in, [128, 128], f16)
            ident = pload("ident", ident_in, [128, 128], f16)
            w0 = pload("w0", w0_in, [N_FEATS, HIDDEN], f16)
            b0 = pload("b0", b0_in, [HIDDEN, 1])
            wl = pload("wl", wl_in, [HIDDEN, N_LAYERS * HIDDEN], f16)
            wout = pload("wout", wout_in, [HIDDEN, N_CLASSES], f16)
            bout = pload("bout", bout_in, [N_CLASSES, 1])
            tloc = pload("tloc", tloc_in, [128, nchk])
            dcol = pload("dcol", dcol_in, [128, nchk])
            dinvo = pload("dinvo", dinvo_in, [128, TPC])
            h0s = P.tile([HIDDEN, NPC_PAD], f32, tag="h0s")
            # node-major hs of the current layer, kept locally for the
            # self-loop diagonal matmul (also the staging buffer for the
            # hs_shard DMA writes)
            hsl = P.tile([128, TPC, HIDDEN], f16, tag="hsl")
            # constant diagonal tiles: dg[t] = ident * (0.9 * dinv_t)
            dg = P.tile([128, TPC, 128], f16, tag="dg")
            for t in range(TPC):
                nc.vector.tensor_scalar(
                    dg[:, t, :], ident[:], dinvo[:, t:t + 1],
                    1.0 - ALPHA, op0=Alu.mult, op1=Alu.mult)

            hs_shard = [
                DR.tile([NPC_PAD, HROW], f16, tag=f"shard{j}",
                        name=f"hs_shard{j}")
                for j in range(N_LAYERS)
            ]
            hs_full = [
                DR.tile([N_PAD, HROW], f16, tag=f"full{j}",
                        addr_space="Shared", name=f"hs_full{j}")
                for j in range(N_LAYERS)
            ]

            def emit_node_tile(ht, tti, t, nxt_shard):
                """Transpose ht[:, tti*128:...] (fp16 [64,128]) to node-major,
                scale by dinv into the persistent hsl buffer, write the
                padded hs row."""
                tp_ps = PSM.tile([128, HIDDEN], f16, tag="tp")
                nc.tensor.transpose(
                    tp_ps[:], ht[:, tti * 128:(tti + 1) * 128],
                    ident[:HIDDEN, :HIDDEN])
                nc.vector.tensor_scalar(hsl[:, t, :], tp_ps[:],
                                        dinvo[:, t:t + 1], None,
                                        op0=Alu.mult)
                nc.sync.dma_start(
                    nxt_shard[t * 128:(t + 1) * 128, 0:HIDDEN], hsl[:, t, :])

            # ---------------- layer 0: h0 = relu(x @ W0 + b0) ----------------
            for bi in range((TPC + SUB_TT - 1) // SUB_TT):
                t0 = bi * SUB_TT
                w = min(SUB_TT, TPC - t0) * 128
                xt_sb = S.tile([N_FEATS, SUB_TT * 128], f16, tag="xt")
                nc.sync.dma_start(
                    xt_sb[:, :w], xt_in[:, t0 * 128:t0 * 128 + w])
                h_ps = PSM.tile([HIDDEN, SUB_TT * 128], f32, tag="dense")
                nc.tensor.matmul(h_ps[:, :w], lhsT=w0[:], rhs=xt_sb[:, :w],
                                 start=True, stop=True)
                h0t = S.tile([HIDDEN, SUB_TT * 128], f16, tag="ht")
                nc.scalar.activation(h0t[:, :w], h_ps[:, :w], Act.Relu,
                                     bias=b0[:])
                nc.vector.tensor_scalar_mul(
                    h0s[:, t0 * 128:t0 * 128 + w], h0t[:, :w], ALPHA)
                for tti in range(w // 128):
                    emit_node_tile(h0t, tti, t0 + tti, hs_shard[0])
            nc.gpsimd.collective_compute(
                "AllGather", Alu.bypass, replica_groups=rg,
                ins=[hs_shard[0].opt()], outs=[hs_full[0].opt()])

            # ---------------- GCNII layers ----------------
            for li in range(N_LAYERS):
                cur = hs_full[li]
                is_last = li == N_LAYERS - 1
                nxt_shard = hs_shard[li + 1] if not is_last else None
                for g in range(ngroups):
                    tts = list(range(g * GROUP_TT,
                                     min((g + 1) * GROUP_TT, TPC)))
                    subs = [tts[i:i + SUB_TT]
                            for i in range(0, len(tts), SUB_TT)]
                    ps_tiles = [PSA.tile([HIDDEN, SUB_TT * 128], f32,
                                         tag="agg", name=f"agg{li}_{g}_{si}")
                                for si in range(len(subs))]
                    # self-loop contributions: agg^T[:, t] += hs_t^T @ dg_t
                    # (hsl holds this layer's hs, written at end of layer-1)
                    for si, stts in enumerate(subs):
                        for tti, t in enumerate(stts):
                            nc.tensor.matmul(
                                ps_tiles[si][:, tti * 128:(tti + 1) * 128],
                                lhsT=hsl[:, t, :], rhs=dg[:, t, :],
                                start=(tti == 0), stop=False,
                                skip_group_check=True)
                    for (kk, s, n) in calls_by_group[g]:
                        idx_t = S.tile([128, n // 16], dt.int16, tag="idxs")
                        nc.sync.dma_start(
                            idx_t[:], idx_in[:, s // 16:(s + n) // 16])
                        msg = MSG.tile([128, n // 128, HROW], f16,
                                       tag="msg")
                        rows_k = min(SRC_CHUNK, N_PAD - kk * SRC_CHUNK)
                        nc.gpsimd.dma_gather(
                            msg[:],
                            cur[kk * SRC_CHUNK:kk * SRC_CHUNK + rows_k, :],
                            idx_t[:], num_idxs=n, num_idxs_reg=n,
                            elem_size=HROW, single_packet=False)
                        for jj in range(n // 128):
                            j = s // 128 + jj
                            t = int(chunk_tt[j])
                            oh = OH.tile([128, 128], f16, tag="oh")
                            nc.vector.tensor_scalar(
                                oh[:], iota[:], tloc[:, j:j + 1],
                                dcol[:, j:j + 1],
                                op0=Alu.is_equal, op1=Alu.mult)
                            si = (t - g * GROUP_TT) // SUB_TT
                            ci = (t % SUB_TT) * 128
                            nc.tensor.matmul(
                                ps_tiles[si][:, ci:ci + 128],
                                lhsT=msg[:, jj, 0:HIDDEN], rhs=oh[:],
                                start=False, stop=bool(last[j]),
                                skip_group_check=True)
                    for si, stts in enumerate(subs):
                        w = len(stts) * 128
                        n0 = stts[0] * 128
                        sup = S.tile([HIDDEN, SUB_TT * 128], f16, tag="sup")
                        nc.vector.tensor_tensor(
                            out=sup[:, :w], in0=ps_tiles[si][:, :w],
                            in1=h0s[:, n0:n0 + w], op=Alu.add)
                        d_ps = PSM.tile([HIDDEN, SUB_TT * 128], f32,
                                        tag="dense")
                        nc.tensor.matmul(
                            d_ps[:, :w],
                            lhsT=wl[:, li * HIDDEN:(li + 1) * HIDDEN],
                            rhs=sup[:, :w], start=True, stop=True)
                        if not is_last:
                            ht = S.tile([HIDDEN, SUB_TT * 128], f16,
                                        tag="ht")
                            nc.scalar.activation(ht[:, :w], d_ps[:, :w],
                                                 Act.Relu)
                            for tti, t in enumerate(stts):
                                emit_node_tile(ht, tti, t, nxt_shard)
                        else:
                            ht = S.tile([HIDDEN, SUB_TT * 128], f16,
                                        tag="ht")
                            nc.scalar.activation(ht[:, :w], d_ps[:, :w],
                                                 Act.Relu)
                            o_ps_full = PSM.tile([HIDDEN, SUB_TT * 128], f32,
                                                 tag="dense")
                            o_ps = o_ps_full[:N_CLASSES, :]
                            nc.tensor.matmul(o_ps[:, :w], lhsT=wout[:],
                                             rhs=ht[:, :w],
                                             start=True, stop=True)
                            o_sb = S.tile([N_CLASSES, SUB_TT * 128], f16,
                                          tag="osb")
                            nc.vector.tensor_scalar(
                                o_sb[:, :w], o_ps[:, :w], bout[:], None,
                                op0=Alu.add)
                            for tti, t in enumerate(stts):
                                tp_ps = PSM.tile([128, HIDDEN], f16,
                                                 tag="tp")
                                nc.tensor.transpose(
                                    tp_ps[:, :N_CLASSES],
                                    o_sb[:, tti * 128:(tti + 1) * 128],
                                    ident[:N_CLASSES, :N_CLASSES])
                                ot = S.tile([128, N_CLASSES], f32, tag="ot")
                                nc.vector.tensor_copy(
                                    ot[:], tp_ps[:, :N_CLASSES])
                                nc.sync.dma_start(
                                    out_dram[t * 128:(t + 1) * 128, :],
                                    ot[:])
                if not is_last:
                    nc.gpsimd.collective_compute(
                        "AllGather", Alu.bypass, replica_groups=rg,
                        ins=[nxt_shard.opt()],
                        outs=[hs_full[li + 1].opt()])

    nc.compile()
    return nc


def kernel(x, edge_index, W0, b0, Wl, W_out, b_out):
    from concourse.bass_utils import run_bass_kernel_spmd

    x = np.asarray(x, dtype=np.float32)
    edge_index = np.asarray(edge_index)
    W0 = np.asarray(W0, dtype=np.float32)
    b0 = np.asarray(b0, dtype=np.float32)
    Wl = np.asarray(Wl, dtype=np.float32)
    W_out = np.asarray(W_out, dtype=np.float32)
    b_out = np.asarray(b_out, dtype=np.float32)

    struct, per_core = _preprocess(edge_index)
    nc = _build_program(struct)

    betas = np.array(
        [math.log(LAMDA / (i + 1) + 1.0) for i in range(N_LAYERS)],
        dtype=np.float32)
    wl_host = np.zeros((HIDDEN, N_LAYERS * HIDDEN), np.float32)
    eye = np.eye(HIDDEN, dtype=np.float32)
    for i in range(N_LAYERS):
        wl_host[:, i * HIDDEN:(i + 1) * HIDDEN] = (
            betas[i] * Wl[i] + (1.0 - betas[i]) * eye)
    iota_host = np.tile(np.arange(128, dtype=np.float32), (128, 1))
    ident_host = np.eye(128, dtype=np.float32)

    in_maps = []
    for c in range(NCORES):
        pc = per_core[c]
        pm = pc["perm"]
        xp = np.zeros((NPC_PAD, N_FEATS), np.float32)
        valid = pm >= 0
        xp[valid] = x[c * NPC:(c + 1) * NPC][pm[valid]]
        in_maps.append({
            "xt": np.ascontiguousarray(xp.T).astype(np.float16),
            "idx": pc["idx"],
            "tloc": pc["tloc"],
            "dcol": pc["dcol"],
            "dinvo": pc["dinvo"],
            "w0": W0.astype(np.float16),
            "b0": b0.reshape(HIDDEN, 1),
            "wl": wl_host.astype(np.float16),
            "wout": W_out.astype(np.float16),
            "bout": b_out.reshape(N_CLASSES, 1),
            "iota": iota_host.astype(np.float16),
            "ident": ident_host.astype(np.float16),
        })

    res = run_bass_kernel_spmd(
        nc, in_maps, core_ids=list(range(NCORES)), trace=TRACE)
    _LAST_RESULT["res"] = res
    out = np.empty((N_NODES, N_CLASSES), np.float32)
    for c in range(NCORES):
        pm = per_core[c]["perm"]
        valid = pm >= 0
        block = res.results[c]["out"]
        out[c * NPC + pm[valid]] = block[valid]
    return out


# revision 18
# speedup vs baseline: 1.5072x; 1.0523x over previous
"""GCNII (nn_GCNII_17626545783193) Bass/Trainium2 kernel, 8 NeuronCores.

Strategy (target-node sharding, feature-major compute, fp16 data path):
  - Nodes sharded 12500/core (padded to 12544 = 98*128). Edges partitioned
    by target core; self-loops handled via per-tile diagonal matmuls against
    a locally kept node-major hs copy (not in the gather stream).
  - Per-core target permutation balances per-(ttile, chunk) edge counts
    across tiles AND cores, minimizing the 128-slot padding of the uniform
    (cross-core identical) edge stream.
  - gcn_norm factored: norm[e] = dinv[src]*dinv[dst]. dinv[src] is folded
    into the gathered features (hs = dinv * h, fp16 [N, 128]-padded rows,
    256B, replicated via Shared-output AllGather each layer); 0.9*dinv[dst]
    is folded into the one-hot scatter matrix.
  - Per layer, per core: dma_gather of in-edge source rows (256B fp16) from
    hs_full; DVE builds one-hot [128edge, 128target] fp16 (iota==tloc)*dcol;
    PE computes agg^T = msgs^T @ onehot accumulated in PSUM [64, 512] fp32;
    support = agg + 0.1*h0 (fp32 h0, fp16 out); GCNII dense update via ONE
    matmul with host-merged W' = beta*Wl + (1-beta)*I; ACT relu; PE transpose
    to node-major; scale by dinv; strided write into the padded hs rows;
    AllGather.
"""
import sys
sys.path.insert(0, "/opt/trn_rl_repo")

import math
import numpy as np

# ---- problem constants (hardcoded per spec) ----
N_NODES = 100000
N_FEATS = 128
HIDDEN = 64
N_CLASSES = 40
N_LAYERS = 8
ALPHA = 0.1
LAMDA = 0.5
NCORES = 8

NPC = N_NODES // NCORES            # 12500 real nodes per core
TPC = (NPC + 127) // 128           # 98 target tiles per core
NPC_PAD = TPC * 128                # 12544
N_PAD = NPC_PAD * NCORES           # 100352
HROW = 128                         # padded fp16 row elems (256B)
GROUP_TT = 8                       # target tiles per gather group
SUB_TT = 4                         # target tiles per psum tile (512 cols)
SRC_CHUNK = 32768                  # int16 index range per gather source chunk
NCHUNKS = (N_PAD + SRC_CHUNK - 1) // SRC_CHUNK  # 4
CALL_MAX = 8192                    # max indices per dma_gather call

TRACE = False          # test.py sets this for profiling
_LAST_RESULT = {}      # test.py reads exec_time from here


def _balance_tiles(deg_ck):
    """Assign 12500 local targets to 98 tiles of <=128, balancing total
    degree: sort by degree desc, snake-deal. Returns perm: perm[new_pos] =
    old local id, laid out tile-major (tile t = perm[t*128:(t+1)*128],
    padded with -1)."""
    tot = deg_ck.sum(axis=1)
    order = np.argsort(-tot, kind="stable")
    tiles = [[] for _ in range(TPC)]
    tsum = np.zeros(TPC)
    # snake deal in rounds of TPC
    i = 0
    fwd = True
    while i < len(order):
        rng = range(TPC) if fwd else range(TPC - 1, -1, -1)
        for t in rng:
            if i >= len(order):
                break
            if len(tiles[t]) < 128:
                tiles[t].append(order[i])
                tsum[t] += tot[order[i]]
                i += 1
        fwd = not fwd
    # order tiles by total degree desc so maxima align across cores
    tile_order = np.argsort(-tsum, kind="stable")
    perm = np.full(NPC_PAD, -1, np.int64)
    for newt, oldt in enumerate(tile_order):
        ids = tiles[oldt]
        perm[newt * 128:newt * 128 + len(ids)] = ids
    return perm


def _preprocess(edge_index):
    """Build the uniform (cross-core identical) edge stream structure and the
    per-core data arrays (with per-core balancing permutations)."""
    row = np.asarray(edge_index[0], dtype=np.int64)
    col = np.asarray(edge_index[1], dtype=np.int64)
    deg = np.bincount(col, minlength=N_NODES).astype(np.float32) + 1.0
    dinv = (1.0 / np.sqrt(deg)).astype(np.float32)

    # self-loops are handled on-chip via a per-tile diagonal matmul against
    # the locally-kept node-major hs copy — they are NOT in the edge stream.
    r = row
    cl = col
    core = cl // NPC
    lcol = cl - core * NPC                       # 0..12499 local target

    # ---- per-core balancing permutation over local targets ----
    src_core = r // NPC
    src_local = r - src_core * NPC
    perms = []            # perm[new_pos (padded)] = old local id
    inv_perms = []        # inv[old local id] = new_pos (padded)
    new_lcol = np.empty_like(lcol)
    for c in range(NCORES):
        m = core == c
        # per-target degree per source chunk (chunk of the *old* padded gp;
        # balancing on totals is enough, chunks are ~proportional)
        deg_c = np.bincount(lcol[m], minlength=NPC)
        perm = _balance_tiles(deg_c.reshape(-1, 1).astype(np.float64))
        inv = np.full(NPC, -1, np.int64)
        valid = perm >= 0
        inv[perm[valid]] = np.nonzero(valid)[0]
        perms.append(perm)
        inv_perms.append(inv)
        new_lcol[m] = inv[lcol[m]]
    lcol = new_lcol                               # padded-permuted local target

    # padded global source id (through the source core's permutation)
    gp = np.empty_like(r)
    for c in range(NCORES):
        m = src_core == c
        gp[m] = c * NPC_PAD + inv_perms[c][src_local[m]]

    tt = lcol >> 7                               # target tile 0..97
    k = gp // SRC_CHUNK                          # source chunk 0..3

    ngroups = (TPC + GROUP_TT - 1) // GROUP_TT

    slot = (core * TPC + tt) * NCHUNKS + k
    counts = np.bincount(slot, minlength=NCORES * TPC * NCHUNKS).reshape(
        NCORES, TPC, NCHUNKS
    )
    maxc = counts.max(axis=0)                    # [TPC, NCHUNKS]
    slot_chunks = -(-maxc // 128)                # ceil; 0 allowed
    slot_len = slot_chunks * 128

    slot_off = np.zeros((TPC, NCHUNKS), np.int64)
    off = 0
    chunk_tt = []
    calls_by_group = [[] for _ in range(ngroups)]
    for g in range(ngroups):
        tts = range(g * GROUP_TT, min((g + 1) * GROUP_TT, TPC))
        for kk in range(NCHUNKS):
            run_start = off
            for t in tts:
                slot_off[t, kk] = off
                off += slot_len[t, kk]
                chunk_tt += [t] * slot_chunks[t, kk]
            s = run_start
            while s < off:
                n = min(CALL_MAX, off - s)
                calls_by_group[g].append((kk, int(s), int(n)))
                s += n
    e_pad = off
    nchk = e_pad // 128
    chunk_tt = np.asarray(chunk_tt, np.int64)
    # start/stop are per PSUM tile (= subgroup of SUB_TT ttiles): exactly one
    # start=True (chronologically first matmul into the bank) and one
    # stop=True (last). A second start into a live bank wipes it (bank-wide
    # pending-zero), so interleaved per-ttile groups are NOT allowed.
    sub_id = chunk_tt // SUB_TT
    first = np.zeros(nchk, bool)
    last = np.zeros(nchk, bool)
    for sg in np.unique(sub_id):
        js = np.nonzero(sub_id == sg)[0]
        first[js[0]] = True
        last[js[-1]] = True

    per_core = []
    for c in range(NCORES):
        m = core == c
        rc = r[m]
        clm = cl[m]
        lc = lcol[m]
        tc_ = tt[m]
        kc = k[m]
        gpc = gp[m]
        skey = tc_ * NCHUNKS + kc
        order = np.argsort(skey, kind="stable")
        sorted_key = skey[order]
        firsts = np.searchsorted(sorted_key, sorted_key, side="left")
        ranks = np.arange(len(sorted_key)) - firsts
        pos = slot_off[tc_[order], kc[order]] + ranks

        idx_stream = np.zeros(e_pad, np.int16)
        tloc_stream = np.zeros(e_pad, np.float32)
        dcol_stream = np.zeros(e_pad, np.float32)
        idx_stream[pos] = (gpc[order] - kc[order] * SRC_CHUNK).astype(np.int16)
        tloc_stream[pos] = (lc[order] & 127).astype(np.float32)
        dcol_stream[pos] = ((1.0 - ALPHA) * dinv[clm[order]]).astype(np.float32)

        idx_w = np.zeros((128, e_pad // 16), np.int16)
        for g in range(ngroups):
            for (kk, s, n) in calls_by_group[g]:
                blk = idx_stream[s:s + n].reshape(n // 16, 16).T
                idx_w[:, s // 16:(s + n) // 16] = np.tile(blk, (8, 1))
        tloc_arr = np.ascontiguousarray(tloc_stream.reshape(nchk, 128).T)
        dcol_arr = np.ascontiguousarray(dcol_stream.reshape(nchk, 128).T)
        dv = np.zeros(NPC_PAD, np.float32)
        pm = perms[c]
        valid = pm >= 0
        dv[valid] = dinv[c * NPC + pm[valid]]
        dinvo = np.ascontiguousarray(dv.reshape(TPC, 128).T)
        per_core.append(
            dict(idx=idx_w, tloc=tloc_arr, dcol=dcol_arr, dinvo=dinvo,
                 perm=pm)
        )

    struct = dict(
        e_pad=e_pad,
        nchk=nchk,
        ngroups=ngroups,
        calls_by_group=calls_by_group,
        chunk_tt=chunk_tt,
        first=first,
        last=last,
    )
    return struct, per_core


def _build_program(struct):
    import concourse.bass as bass
    import concourse.mybir as mybir
    import concourse.tile as tile
    import concourse.bacc as bacc

    dt = mybir.dt
    f32 = dt.float32
    f16 = dt.float16
    Alu = mybir.AluOpType
    Act = mybir.ActivationFunctionType

    e_pad = struct["e_pad"]
    nchk = struct["nchk"]
    ngroups = struct["ngroups"]
    calls_by_group = struct["calls_by_group"]
    chunk_tt = struct["chunk_tt"]
    first = struct["first"]
    last = struct["last"]
    rg = [list(range(NCORES))]

    nc = bacc.Bacc("TRN2", target_bir_lowering=False, debug=False,
                   num_devices=NCORES, num_swdge_queues=4)

    xt_in = nc.dram_tensor("xt", [N_FEATS, NPC_PAD], f16, kind="ExternalInput")
    idx_in = nc.dram_tensor("idx", [128, e_pad // 16], dt.int16,
                            kind="ExternalInput")
    tloc_in = nc.dram_tensor("tloc", [128, nchk], f32, kind="ExternalInput")
    dcol_in = nc.dram_tensor("dcol", [128, nchk], f32, kind="ExternalInput")
    dinvo_in = nc.dram_tensor("dinvo", [128, TPC], f32, kind="ExternalInput")
    w0_in = nc.dram_tensor("w0", [N_FEATS, HIDDEN], f16, kind="ExternalInput")
    b0_in = nc.dram_tensor("b0", [HIDDEN, 1], f32, kind="ExternalInput")
    wl_in = nc.dram_tensor("wl", [HIDDEN, N_LAYERS * HIDDEN], f16,
                           kind="ExternalInput")
    wout_in = nc.dram_tensor("wout", [HIDDEN, N_CLASSES], f16,
                             kind="ExternalInput")
    bout_in = nc.dram_tensor("bout", [N_CLASSES, 1], f32, kind="ExternalInput")
    iota_in = nc.dram_tensor("iota", [128, 128], f16, kind="ExternalInput")
    ident_in = nc.dram_tensor("ident", [128, 128], f16, kind="ExternalInput")
    out_dram = nc.dram_tensor("out", [NPC_PAD, N_CLASSES], f32,
                              kind="ExternalOutput")

    with tile.TileContext(nc) as tc:
        with (
            tc.tile_pool(name="persist", bufs=1) as P,
            tc.tile_pool(name="work", bufs=3) as S,
            tc.tile_pool(name="msgp", bufs=4) as MSG,
            tc.tile_pool(name="ohp", bufs=16) as OH,
            tc.tile_pool(name="psagg", bufs=4, space="PSUM") as PSA,
            tc.tile_pool(name="psmisc", bufs=2, space="PSUM") as PSM,
            tc.tile_pool(name="dram", bufs=1, space="DRAM") as DR,
        ):
            def pload(name, src, shape, dtype=f32):
                t = P.tile(shape, dtype, tag=name)
                nc.sync.dma_start(t[:], src[:])
                return t

            iota = pload("iota", iota_in, [128, 128], f16)
            ident = pload("ident", ident_in, [128, 128], f16)
            w0 = pload("w0", w0_in, [N_FEATS, HIDDEN], f16)
            b0 = pload("b0", b0_in, [HIDDEN, 1])
            wl = pload("wl", wl_in, [HIDDEN, N_LAYERS * HIDDEN], f16)
            wout = pload("wout", wout_in, [HIDDEN, N_CLASSES], f16)
            bout = pload("bout", bout_in, [N_CLASSES, 1])
            tloc = pload("tloc", tloc_in, [128, nchk])
            dcol = pload("dcol", dcol_in, [128, nchk])
            dinvo = pload("dinvo", dinvo_in, [128, TPC])
            # layer-invariant gather indices, resident in SBUF
            idxs = pload("idxs", idx_in, [128, e_pad // 16], dt.int16)
            h0s = P.tile([HIDDEN, NPC_PAD], f32, tag="h0s")
            # node-major hs of the current layer, kept locally for the
            # self-loop diagonal matmul (also the staging buffer for the
            # hs_shard DMA writes)
            hsl = P.tile([128, TPC, HIDDEN], f16, tag="hsl")
            # constant diagonal tiles: dg[t] = ident * (0.9 * dinv_t)
            dg = P.tile([128, TPC, 128], f16, tag="dg")
            for t in range(TPC):
                nc.vector.tensor_scalar(
                    dg[:, t, :], ident[:], dinvo[:, t:t + 1],
                    1.0 - ALPHA, op0=Alu.mult, op1=Alu.mult)

            hs_shard = [
                DR.tile([NPC_PAD, HROW], f16, tag=f"shard{j}",
                        name=f"hs_shard{j}")
                for j in range(N_LAYERS)
            ]
            hs_full = [
                DR.tile([N_PAD, HROW], f16, tag=f"full{j}",
                        addr_space="Shared", name=f"hs_full{j}")
                for j in range(N_LAYERS)
            ]

            def emit_node_tile(ht, tti, t, nxt_shard):
                """Transpose ht[:, tti*128:...] (fp16 [64,128]) to node-major,
                scale by dinv into the persistent hsl buffer, write the
                padded hs row."""
                tp_ps = PSM.tile([128, HIDDEN], f16, tag="tp")
                nc.tensor.transpose(
                    tp_ps[:], ht[:, tti * 128:(tti + 1) * 128],
                    ident[:HIDDEN, :HIDDEN])
                nc.vector.tensor_scalar(hsl[:, t, :], tp_ps[:],
                                        dinvo[:, t:t + 1], None,
                                        op0=Alu.mult)
                nc.sync.dma_start(
                    nxt_shard[t * 128:(t + 1) * 128, 0:HIDDEN], hsl[:, t, :])

            # ---------------- layer 0: h0 = relu(x @ W0 + b0) ----------------
            for bi in range((TPC + SUB_TT - 1) // SUB_TT):
                t0 = bi * SUB_TT
                w = min(SUB_TT, TPC - t0) * 128
                xt_sb = S.tile([N_FEATS, SUB_TT * 128], f16, tag="xt")
                nc.sync.dma_start(
                    xt_sb[:, :w], xt_in[:, t0 * 128:t0 * 128 + w])
                h_ps = PSM.tile([HIDDEN, SUB_TT * 128], f32, tag="dense")
                nc.tensor.matmul(h_ps[:, :w], lhsT=w0[:], rhs=xt_sb[:, :w],
                                 start=True, stop=True)
                h0t = S.tile([HIDDEN, SUB_TT * 128], f16, tag="ht")
                nc.scalar.activation(h0t[:, :w], h_ps[:, :w], Act.Relu,
                                     bias=b0[:])
                nc.vector.tensor_scalar_mul(
                    h0s[:, t0 * 128:t0 * 128 + w], h0t[:, :w], ALPHA)
                for tti in range(w // 128):
                    emit_node_tile(h0t, tti, t0 + tti, hs_shard[0])
            nc.gpsimd.collective_compute(
                "AllGather", Alu.bypass, replica_groups=rg,
                ins=[hs_shard[0].opt()], outs=[hs_full[0].opt()])

            # ---------------- GCNII layers ----------------
            qctr = [0]
            for li in range(N_LAYERS):
                cur = hs_full[li]
                is_last = li == N_LAYERS - 1
                nxt_shard = hs_shard[li + 1] if not is_last else None
                for g in range(ngroups):
                    tts = list(range(g * GROUP_TT,
                                     min((g + 1) * GROUP_TT, TPC)))
                    subs = [tts[i:i + SUB_TT]
                            for i in range(0, len(tts), SUB_TT)]
                    ps_tiles = [PSA.tile([HIDDEN, SUB_TT * 128], f32,
                                         tag="agg", name=f"agg{li}_{g}_{si}")
                                for si in range(len(subs))]
                    # self-loop contributions: agg^T[:, t] += hs_t^T @ dg_t
                    # (hsl holds this layer's hs, written at end of layer-1)
                    for si, stts in enumerate(subs):
                        for tti, t in enumerate(stts):
                            nc.tensor.matmul(
                                ps_tiles[si][:, tti * 128:(tti + 1) * 128],
                                lhsT=hsl[:, t, :], rhs=dg[:, t, :],
                                start=(tti == 0), stop=False,
                                skip_group_check=True)
                    for (kk, s, n) in calls_by_group[g]:
                        msg = MSG.tile([128, n // 128, HROW], f16,
                                       tag="msg")
                        rows_k = min(SRC_CHUNK, N_PAD - kk * SRC_CHUNK)
                        nc.gpsimd.dma_gather(
                            msg[:],
                            cur[kk * SRC_CHUNK:kk * SRC_CHUNK + rows_k, :],
                            idxs[:, s // 16:(s + n) // 16],
                            num_idxs=n, num_idxs_reg=n,
                            elem_size=HROW, single_packet=False,
                            queue_num=qctr[0] % 4)
                        qctr[0] += 1
                        for jj in range(n // 128):
                            j = s // 128 + jj
                            t = int(chunk_tt[j])
                            oh = OH.tile([128, 128], f16, tag="oh")
                            nc.vector.tensor_scalar(
                                oh[:], iota[:], tloc[:, j:j + 1],
                                dcol[:, j:j + 1],
                                op0=Alu.is_equal, op1=Alu.mult)
                            si = (t - g * GROUP_TT) // SUB_TT
                            ci = (t % SUB_TT) * 128
                            nc.tensor.matmul(
                                ps_tiles[si][:, ci:ci + 128],
                                lhsT=msg[:, jj, 0:HIDDEN], rhs=oh[:],
                                start=False, stop=bool(last[j]),
                                skip_group_check=True)
                    for si, stts in enumerate(subs):
                        w = len(stts) * 128
                        n0 = stts[0] * 128
                        sup = S.tile([HIDDEN, SUB_TT * 128], f16, tag="sup")
                        nc.vector.tensor_tensor(
                            out=sup[:, :w], in0=ps_tiles[si][:, :w],
                            in1=h0s[:, n0:n0 + w], op=Alu.add)
                        d_ps = PSM.tile([HIDDEN, SUB_TT * 128], f32,
                                        tag="dense")
                        nc.tensor.matmul(
                            d_ps[:, :w],
                            lhsT=wl[:, li * HIDDEN:(li + 1) * HIDDEN],
                            rhs=sup[:, :w], start=True, stop=True)
                        if not is_last:
                            ht = S.tile([HIDDEN, SUB_TT * 128], f16,
                                        tag="ht")
                            nc.scalar.activation(ht[:, :w], d_ps[:, :w],
                                                 Act.Relu)
                            for tti, t in enumerate(stts):
                                emit_node_tile(ht, tti, t, nxt_shard)
                        else:
                            ht = S.tile([HIDDEN, SUB_TT * 128], f16,
                                        tag="ht")
                            nc.scalar.activation(ht[:, :w], d_ps[:, :w],
                                                 Act.Relu)
                            o_ps_full = PSM.tile([HIDDEN, SUB_TT * 128], f32,
                                                 tag="dense")
                            o_ps = o_ps_full[:N_CLASSES, :]
                            nc.tensor.matmul(o_ps[:, :w], lhsT=wout[:],
                                             rhs=ht[:, :w],
                                             start=True, stop=True)
                            o_sb = S.tile([N_CLASSES, SUB_TT * 128], f16,
                                          tag="osb")
                            nc.vector.tensor_scalar(
                                o_sb[:, :w], o_ps[:, :w], bout[:], None,
                                op0=Alu.add)
                            for tti, t in enumerate(stts):
                                tp_ps = PSM.tile([128, HIDDEN], f16,
                                                 tag="tp")
                                nc.tensor.transpose(
                                    tp_ps[:, :N_CLASSES],
                                    o_sb[:, tti * 128:(tti + 1) * 128],
                                    ident[:N_CLASSES, :N_CLASSES])
                                ot = S.tile([128, N_CLASSES], f32, tag="ot")
                                nc.vector.tensor_copy(
                                    ot[:], tp_ps[:, :N_CLASSES])
                                nc.sync.dma_start(
                                    out_dram[t * 128:(t + 1) * 128, :],
                                    ot[:])
                if not is_last:
                    nc.gpsimd.collective_compute(
                        "AllGather", Alu.bypass, replica_groups=rg,
                        ins=[nxt_shard.opt()],
                        outs=[hs_full[li + 1].opt()])

    nc.compile()
    return nc


def kernel(x, edge_index, W0, b0, Wl, W_out, b_out):
    from concourse.bass_utils import run_bass_kernel_spmd

    x = np.asarray(x, dtype=np.float32)
    edge_index = np.asarray(edge_index)
    W0 = np.asarray(W0, dtype=np.float32)
    b0 = np.asarray(b0, dtype=np.float32)
    Wl = np.asarray(Wl, dtype=np.float32)
    W_out = np.asarray(W_out, dtype=np.float32)
    b_out = np.asarray(b_out, dtype=np.float32)

    struct, per_core = _preprocess(edge_index)
    nc = _build_program(struct)

    betas = np.array(
        [math.log(LAMDA / (i + 1) + 1.0) for i in range(N_LAYERS)],
        dtype=np.float32)
    wl_host = np.zeros((HIDDEN, N_LAYERS * HIDDEN), np.float32)
    eye = np.eye(HIDDEN, dtype=np.float32)
    for i in range(N_LAYERS):
        wl_host[:, i * HIDDEN:(i + 1) * HIDDEN] = (
            betas[i] * Wl[i] + (1.0 - betas[i]) * eye)
    iota_host = np.tile(np.arange(128, dtype=np.float32), (128, 1))
    ident_host = np.eye(128, dtype=np.float32)

    in_maps = []
    for c in range(NCORES):
        pc = per_core[c]
        pm = pc["perm"]
        xp = np.zeros((NPC_PAD, N_FEATS), np.float32)
        valid = pm >= 0
        xp[valid] = x[c * NPC:(c + 1) * NPC][pm[valid]]
        in_maps.append({
            "xt": np.ascontiguousarray(xp.T).astype(np.float16),
            "idx": pc["idx"],
            "tloc": pc["tloc"],
            "dcol": pc["dcol"],
            "dinvo": pc["dinvo"],
            "w0": W0.astype(np.float16),
            "b0": b0.reshape(HIDDEN, 1),
            "wl": wl_host.astype(np.float16),
            "wout": W_out.astype(np.float16),
            "bout": b_out.reshape(N_CLASSES, 1),
            "iota": iota_host.astype(np.float16),
            "ident": ident_host.astype(np.float16),
        })

    res = run_bass_kernel_spmd(
        nc, in_maps, core_ids=list(range(NCORES)), trace=TRACE)
    _LAST_RESULT["res"] = res
    out = np.empty((N_NODES, N_CLASSES), np.float32)
    for c in range(NCORES):
        pm = per_core[c]["perm"]
        valid = pm >= 0
        block = res.results[c]["out"]
        out[c * NPC + pm[valid]] = block[valid]
    return out


# revision 23
# speedup vs baseline: 1.6583x; 1.1003x over previous
"""GCNII (nn_GCNII_17626545783193) Bass/Trainium2 kernel, 8 NeuronCores.

Strategy (target-node sharding, feature-major compute, fp16 data path):
  - Nodes sharded 12500/core (padded to 12544 = 98*128). Edges partitioned
    by target core; self-loops handled via per-tile diagonal matmuls against
    a locally kept node-major hs copy (not in the gather stream).
  - Per-core target permutation balances per-(ttile, chunk) edge counts
    across tiles AND cores, minimizing the 128-slot padding of the uniform
    (cross-core identical) edge stream.
  - gcn_norm factored: norm[e] = dinv[src]*dinv[dst]. dinv[src] is folded
    into the gathered features (hs = dinv * h, fp16 [N, 128]-padded rows,
    256B, replicated via Shared-output AllGather each layer); 0.9*dinv[dst]
    is folded into the one-hot scatter matrix.
  - Per layer, per core: dma_gather of in-edge source rows (256B fp16) from
    hs_full; DVE builds one-hot [128edge, 128target] fp16 (iota==tloc)*dcol;
    PE computes agg^T = msgs^T @ onehot accumulated in PSUM [64, 512] fp32;
    support = agg + 0.1*h0 (fp32 h0, fp16 out); GCNII dense update via ONE
    matmul with host-merged W' = beta*Wl + (1-beta)*I; ACT relu; PE transpose
    to node-major; scale by dinv; strided write into the padded hs rows;
    AllGather.
"""
import sys
sys.path.insert(0, "/opt/trn_rl_repo")

import math
import numpy as np

# ---- problem constants (hardcoded per spec) ----
N_NODES = 100000
N_FEATS = 128
HIDDEN = 64
N_CLASSES = 40
N_LAYERS = 8
ALPHA = 0.1
LAMDA = 0.5
NCORES = 8

NPC = N_NODES // NCORES            # 12500 real nodes per core
TPC = (NPC + 127) // 128           # 98 target tiles per core
NPC_PAD = TPC * 128                # 12544
N_PAD = NPC_PAD * NCORES           # 100352
HROW = 128                         # padded fp16 row elems (256B)
GROUP_TT = 8                       # target tiles per gather group
SUB_TT = 4                         # target tiles per psum tile (512 cols)
SRC_CHUNK = 32768                  # int16 index range per gather source chunk
NCHUNKS = (N_PAD + SRC_CHUNK - 1) // SRC_CHUNK  # 4
CALL_MAX = 8192                    # max indices per dma_gather call

TRACE = False          # test.py sets this for profiling
_LAST_RESULT = {}      # test.py reads exec_time from here


def _balance_tiles(deg_ck):
    """Assign 12500 local targets to 98 tiles of <=128, balancing total
    degree: sort by degree desc, snake-deal. Returns perm: perm[new_pos] =
    old local id, laid out tile-major (tile t = perm[t*128:(t+1)*128],
    padded with -1)."""
    tot = deg_ck.sum(axis=1)
    order = np.argsort(-tot, kind="stable")
    tiles = [[] for _ in range(TPC)]
    tsum = np.zeros(TPC)
    # snake deal in rounds of TPC
    i = 0
    fwd = True
    while i < len(order):
        rng = range(TPC) if fwd else range(TPC - 1, -1, -1)
        for t in rng:
            if i >= len(order):
                break
            if len(tiles[t]) < 128:
                tiles[t].append(order[i])
                tsum[t] += tot[order[i]]
                i += 1
        fwd = not fwd
    # order tiles by total degree desc so maxima align across cores
    tile_order = np.argsort(-tsum, kind="stable")
    perm = np.full(NPC_PAD, -1, np.int64)
    for newt, oldt in enumerate(tile_order):
        ids = tiles[oldt]
        perm[newt * 128:newt * 128 + len(ids)] = ids
    return perm


def _preprocess(edge_index):
    """Build the uniform (cross-core identical) edge stream structure and the
    per-core data arrays (with per-core balancing permutations)."""
    row = np.asarray(edge_index[0], dtype=np.int64)
    col = np.asarray(edge_index[1], dtype=np.int64)
    deg = np.bincount(col, minlength=N_NODES).astype(np.float32) + 1.0
    dinv = (1.0 / np.sqrt(deg)).astype(np.float32)

    # self-loops are handled on-chip via a per-tile diagonal matmul against
    # the locally-kept node-major hs copy — they are NOT in the edge stream.
    r = row
    cl = col
    core = cl // NPC
    lcol = cl - core * NPC                       # 0..12499 local target

    # ---- per-core balancing permutation over local targets ----
    src_core = r // NPC
    src_local = r - src_core * NPC
    perms = []            # perm[new_pos (padded)] = old local id
    inv_perms = []        # inv[old local id] = new_pos (padded)
    new_lcol = np.empty_like(lcol)
    for c in range(NCORES):
        m = core == c
        # per-target degree per source chunk (chunk of the *old* padded gp;
        # balancing on totals is enough, chunks are ~proportional)
        deg_c = np.bincount(lcol[m], minlength=NPC)
        perm = _balance_tiles(deg_c.reshape(-1, 1).astype(np.float64))
        inv = np.full(NPC, -1, np.int64)
        valid = perm >= 0
        inv[perm[valid]] = np.nonzero(valid)[0]
        perms.append(perm)
        inv_perms.append(inv)
        new_lcol[m] = inv[lcol[m]]
    lcol = new_lcol                               # padded-permuted local target

    # padded global source id (through the source core's permutation)
    gp = np.empty_like(r)
    for c in range(NCORES):
        m = src_core == c
        gp[m] = c * NPC_PAD + inv_perms[c][src_local[m]]

    # pair-granular slots: each 128-edge chunk targets one PAIR of ttiles
    # (256 targets) via a [128, 256] one-hot.
    NPAIR = TPC // 2                             # 49
    pr = lcol >> 8                               # target pair 0..48
    k = gp // SRC_CHUNK                          # source chunk 0..3

    ngroups = (TPC + GROUP_TT - 1) // GROUP_TT

    slot = (core * NPAIR + pr) * NCHUNKS + k
    counts = np.bincount(slot, minlength=NCORES * NPAIR * NCHUNKS).reshape(
        NCORES, NPAIR, NCHUNKS
    )
    maxc = counts.max(axis=0)                    # [NPAIR, NCHUNKS]
    slot_chunks = -(-maxc // 128)                # ceil; 0 allowed
    slot_len = slot_chunks * 128

    GROUP_PR = GROUP_TT // 2
    slot_off = np.zeros((NPAIR, NCHUNKS), np.int64)
    off = 0
    chunk_pr = []
    calls_by_group = [[] for _ in range(ngroups)]
    for g in range(ngroups):
        prs = range(g * GROUP_PR, min((g + 1) * GROUP_PR, NPAIR))
        for kk in range(NCHUNKS):
            run_start = off
            for t in prs:
                slot_off[t, kk] = off
                off += slot_len[t, kk]
                chunk_pr += [t] * slot_chunks[t, kk]
            s = run_start
            while s < off:
                n = min(CALL_MAX, off - s)
                calls_by_group[g].append((kk, int(s), int(n)))
                s += n
    e_pad = off
    nchk = e_pad // 128
    chunk_pr = np.asarray(chunk_pr, np.int64)
    # start/stop are per PSUM tile (= subgroup of SUB_TT//2 pairs): exactly
    # one start=True (chronologically first matmul into the bank) and one
    # stop=True (last).
    sub_id = chunk_pr // (SUB_TT // 2)
    first = np.zeros(nchk, bool)
    last = np.zeros(nchk, bool)
    for sg in np.unique(sub_id):
        js = np.nonzero(sub_id == sg)[0]
        first[js[0]] = True
        last[js[-1]] = True

    per_core = []
    for c in range(NCORES):
        m = core == c
        rc = r[m]
        clm = cl[m]
        lc = lcol[m]
        tc_ = pr[m]
        kc = k[m]
        gpc = gp[m]
        skey = tc_ * NCHUNKS + kc
        order = np.argsort(skey, kind="stable")
        sorted_key = skey[order]
        firsts = np.searchsorted(sorted_key, sorted_key, side="left")
        ranks = np.arange(len(sorted_key)) - firsts
        pos = slot_off[tc_[order], kc[order]] + ranks

        idx_stream = np.zeros(e_pad, np.int16)
        tloc_stream = np.zeros(e_pad, np.float32)
        dcol_stream = np.zeros(e_pad, np.float32)
        idx_stream[pos] = (gpc[order] - kc[order] * SRC_CHUNK).astype(np.int16)
        tloc_stream[pos] = (lc[order] & 255).astype(np.float32)
        dcol_stream[pos] = ((1.0 - ALPHA) * dinv[clm[order]]).astype(np.float32)

        idx_w = np.zeros((128, e_pad // 16), np.int16)
        for g in range(ngroups):
            for (kk, s, n) in calls_by_group[g]:
                blk = idx_stream[s:s + n].reshape(n // 16, 16).T
                idx_w[:, s // 16:(s + n) // 16] = np.tile(blk, (8, 1))
        tloc_arr = np.ascontiguousarray(tloc_stream.reshape(nchk, 128).T)
        dcol_arr = np.ascontiguousarray(dcol_stream.reshape(nchk, 128).T)
        dv = np.zeros(NPC_PAD, np.float32)
        pm = perms[c]
        valid = pm >= 0
        dv[valid] = dinv[c * NPC + pm[valid]]
        dinvo = np.ascontiguousarray(dv.reshape(TPC, 128).T)
        per_core.append(
            dict(idx=idx_w, tloc=tloc_arr, dcol=dcol_arr, dinvo=dinvo,
                 perm=pm)
        )

    struct = dict(
        e_pad=e_pad,
        nchk=nchk,
        ngroups=ngroups,
        calls_by_group=calls_by_group,
        chunk_pr=chunk_pr,
        first=first,
        last=last,
    )
    return struct, per_core


def _build_program(struct):
    import concourse.bass as bass
    import concourse.mybir as mybir
    import concourse.tile as tile
    import concourse.bacc as bacc

    dt = mybir.dt
    f32 = dt.float32
    f16 = dt.float16
    Alu = mybir.AluOpType
    Act = mybir.ActivationFunctionType

    e_pad = struct["e_pad"]
    nchk = struct["nchk"]
    ngroups = struct["ngroups"]
    calls_by_group = struct["calls_by_group"]
    chunk_pr = struct["chunk_pr"]
    first = struct["first"]
    last = struct["last"]
    rg = [list(range(NCORES))]

    nc = bacc.Bacc("TRN2", target_bir_lowering=False, debug=False,
                   num_devices=NCORES, num_swdge_queues=4)

    xt_in = nc.dram_tensor("xt", [N_FEATS, NPC_PAD], f16, kind="ExternalInput")
    idx_in = nc.dram_tensor("idx", [128, e_pad // 16], dt.int16,
                            kind="ExternalInput")
    tloc_in = nc.dram_tensor("tloc", [128, nchk], f32, kind="ExternalInput")
    dcol_in = nc.dram_tensor("dcol", [128, nchk], f32, kind="ExternalInput")
    dinvo_in = nc.dram_tensor("dinvo", [128, TPC], f32, kind="ExternalInput")
    w0_in = nc.dram_tensor("w0", [N_FEATS, HIDDEN], f16, kind="ExternalInput")
    b0_in = nc.dram_tensor("b0", [HIDDEN, 1], f32, kind="ExternalInput")
    wl_in = nc.dram_tensor("wl", [HIDDEN, N_LAYERS * HIDDEN], f16,
                           kind="ExternalInput")
    wout_in = nc.dram_tensor("wout", [HIDDEN, N_CLASSES], f16,
                             kind="ExternalInput")
    bout_in = nc.dram_tensor("bout", [N_CLASSES, 1], f32, kind="ExternalInput")
    iota_in = nc.dram_tensor("iota", [128, 256], f16, kind="ExternalInput")
    ident_in = nc.dram_tensor("ident", [128, 128], f16, kind="ExternalInput")
    out_dram = nc.dram_tensor("out", [NPC_PAD, N_CLASSES], f32,
                              kind="ExternalOutput")

    with tile.TileContext(nc) as tc:
        with (
            tc.tile_pool(name="persist", bufs=1) as P,
            tc.tile_pool(name="work", bufs=3) as S,
            tc.tile_pool(name="msgp", bufs=5) as MSG,
            tc.tile_pool(name="ohp", bufs=12) as OH,
            tc.tile_pool(name="psagg", bufs=4, space="PSUM") as PSA,
            tc.tile_pool(name="psmisc", bufs=2, space="PSUM") as PSM,
            tc.tile_pool(name="dram", bufs=1, space="DRAM") as DR,
        ):
            def pload(name, src, shape, dtype=f32):
                t = P.tile(shape, dtype, tag=name)
                nc.sync.dma_start(t[:], src[:])
                return t

            iota = pload("iota", iota_in, [128, 256], f16)
            ident = pload("ident", ident_in, [128, 128], f16)
            w0 = pload("w0", w0_in, [N_FEATS, HIDDEN], f16)
            b0 = pload("b0", b0_in, [HIDDEN, 1])
            wl = pload("wl", wl_in, [HIDDEN, N_LAYERS * HIDDEN], f16)
            wout = pload("wout", wout_in, [HIDDEN, N_CLASSES], f16)
            bout = pload("bout", bout_in, [N_CLASSES, 1])
            tloc = pload("tloc", tloc_in, [128, nchk])
            dcol = pload("dcol", dcol_in, [128, nchk])
            dinvo = pload("dinvo", dinvo_in, [128, TPC])
            # layer-invariant gather indices, resident in SBUF
            idxs = pload("idxs", idx_in, [128, e_pad // 16], dt.int16)
            h0s = P.tile([HIDDEN, NPC_PAD], f32, tag="h0s")
            # node-major hs of the current layer, kept locally for the
            # self-loop diagonal matmul (also the staging buffer for the
            # hs_shard DMA writes)
            hsl = P.tile([128, TPC, HIDDEN], f16, tag="hsl")
            # constant diagonal tiles: dg[t] = ident * (0.9 * dinv_t)
            dg = P.tile([128, TPC, 128], f16, tag="dg")
            for t in range(TPC):
                nc.vector.tensor_scalar(
                    dg[:, t, :], ident[:], dinvo[:, t:t + 1],
                    1.0 - ALPHA, op0=Alu.mult, op1=Alu.mult)

            hs_shard = [
                DR.tile([NPC_PAD, HROW], f16, tag=f"shard{j}",
                        name=f"hs_shard{j}")
                for j in range(N_LAYERS)
            ]
            hs_full = [
                DR.tile([N_PAD, HROW], f16, tag=f"full{j}",
                        addr_space="Shared", name=f"hs_full{j}")
                for j in range(N_LAYERS)
            ]

            def emit_node_tile(ht, tti, t, nxt_shard):
                """Transpose ht[:, tti*128:...] (fp16 [64,128]) to node-major,
                scale by dinv into the persistent hsl buffer, write the
                padded hs row."""
                tp_ps = PSM.tile([128, HIDDEN], f16, tag="tp")
                nc.tensor.transpose(
                    tp_ps[:], ht[:, tti * 128:(tti + 1) * 128],
                    ident[:HIDDEN, :HIDDEN])
                nc.vector.tensor_scalar(hsl[:, t, :], tp_ps[:],
                                        dinvo[:, t:t + 1], None,
                                        op0=Alu.mult)
                nc.sync.dma_start(
                    nxt_shard[t * 128:(t + 1) * 128, 0:HIDDEN], hsl[:, t, :])

            # ---------------- layer 0: h0 = relu(x @ W0 + b0) ----------------
            for bi in range((TPC + SUB_TT - 1) // SUB_TT):
                t0 = bi * SUB_TT
                w = min(SUB_TT, TPC - t0) * 128
                xt_sb = S.tile([N_FEATS, SUB_TT * 128], f16, tag="xt")
                nc.sync.dma_start(
                    xt_sb[:, :w], xt_in[:, t0 * 128:t0 * 128 + w])
                h_ps = PSM.tile([HIDDEN, SUB_TT * 128], f32, tag="dense")
                nc.tensor.matmul(h_ps[:, :w], lhsT=w0[:], rhs=xt_sb[:, :w],
                                 start=True, stop=True)
                h0t = S.tile([HIDDEN, SUB_TT * 128], f16, tag="ht")
                nc.scalar.activation(h0t[:, :w], h_ps[:, :w], Act.Relu,
                                     bias=b0[:])
                nc.vector.tensor_scalar_mul(
                    h0s[:, t0 * 128:t0 * 128 + w], h0t[:, :w], ALPHA)
                for tti in range(w // 128):
                    emit_node_tile(h0t, tti, t0 + tti, hs_shard[0])
            nc.gpsimd.collective_compute(
                "AllGather", Alu.bypass, replica_groups=rg,
                ins=[hs_shard[0].opt()], outs=[hs_full[0].opt()])

            # ---------------- GCNII layers ----------------
            qctr = [0]
            nreg = {}
            for g in range(ngroups):
                for (kk, s_, n) in calls_by_group[g]:
                    if n not in nreg:
                        nreg[n] = nc.gpsimd.to_reg(n)
            for li in range(N_LAYERS):
                cur = hs_full[li]
                is_last = li == N_LAYERS - 1
                nxt_shard = hs_shard[li + 1] if not is_last else None
                for g in range(ngroups):
                    tts = list(range(g * GROUP_TT,
                                     min((g + 1) * GROUP_TT, TPC)))
                    subs = [tts[i:i + SUB_TT]
                            for i in range(0, len(tts), SUB_TT)]
                    ps_tiles = [PSA.tile([HIDDEN, SUB_TT * 128], f32,
                                         tag="agg", name=f"agg{li}_{g}_{si}")
                                for si in range(len(subs))]
                    # self-loop contributions: agg^T[:, t] += hs_t^T @ dg_t
                    # (hsl holds this layer's hs, written at end of layer-1)
                    for si, stts in enumerate(subs):
                        for tti, t in enumerate(stts):
                            nc.tensor.matmul(
                                ps_tiles[si][:, tti * 128:(tti + 1) * 128],
                                lhsT=hsl[:, t, :], rhs=dg[:, t, :],
                                start=(tti == 0), stop=False,
                                skip_group_check=True)
                    for (kk, s, n) in calls_by_group[g]:
                        msg = MSG.tile([128, n // 128, HROW], f16,
                                       tag="msg")
                        rows_k = min(SRC_CHUNK, N_PAD - kk * SRC_CHUNK)
                        nc.gpsimd.dma_gather(
                            msg[:],
                            cur[kk * SRC_CHUNK:kk * SRC_CHUNK + rows_k, :],
                            idxs[:, s // 16:(s + n) // 16],
                            num_idxs=n, num_idxs_reg=nreg[n],
                            elem_size=HROW, single_packet=False,
                            queue_num=qctr[0] % 4)
                        qctr[0] += 1
                        for jj in range(n // 128):
                            j = s // 128 + jj
                            pj = int(chunk_pr[j])
                            oh = OH.tile([128, 256], f16, tag="oh")
                            nc.vector.tensor_scalar(
                                oh[:], iota[:], tloc[:, j:j + 1],
                                dcol[:, j:j + 1],
                                op0=Alu.is_equal, op1=Alu.mult)
                            si = (pj - g * (GROUP_TT // 2)) // (SUB_TT // 2)
                            ci = (pj % (SUB_TT // 2)) * 256
                            nc.tensor.matmul(
                                ps_tiles[si][:, ci:ci + 256],
                                lhsT=msg[:, jj, 0:HIDDEN], rhs=oh[:],
                                start=False, stop=bool(last[j]),
                                skip_group_check=True)
                    for si, stts in enumerate(subs):
                        w = len(stts) * 128
                        n0 = stts[0] * 128
                        sup = S.tile([HIDDEN, SUB_TT * 128], f16, tag="sup")
                        nc.vector.tensor_tensor(
                            out=sup[:, :w], in0=ps_tiles[si][:, :w],
                            in1=h0s[:, n0:n0 + w], op=Alu.add)
                        d_ps = PSM.tile([HIDDEN, SUB_TT * 128], f32,
                                        tag="dense")
                        nc.tensor.matmul(
                            d_ps[:, :w],
                            lhsT=wl[:, li * HIDDEN:(li + 1) * HIDDEN],
                            rhs=sup[:, :w], start=True, stop=True)
                        if not is_last:
                            ht = S.tile([HIDDEN, SUB_TT * 128], f16,
                                        tag="ht")
                            nc.scalar.activation(ht[:, :w], d_ps[:, :w],
                                                 Act.Relu)
                            for tti, t in enumerate(stts):
                                emit_node_tile(ht, tti, t, nxt_shard)
                        else:
                            ht = S.tile([HIDDEN, SUB_TT * 128], f16,
                                        tag="ht")
                            nc.scalar.activation(ht[:, :w], d_ps[:, :w],
                                                 Act.Relu)
                            o_ps_full = PSM.tile([HIDDEN, SUB_TT * 128], f32,
                                                 tag="dense")
                            o_ps = o_ps_full[:N_CLASSES, :]
                            nc.tensor.matmul(o_ps[:, :w], lhsT=wout[:],
                                             rhs=ht[:, :w],
                                             start=True, stop=True)
                            o_sb = S.tile([N_CLASSES, SUB_TT * 128], f16,
                                          tag="osb")
                            nc.vector.tensor_scalar(
                                o_sb[:, :w], o_ps[:, :w], bout[:], None,
                                op0=Alu.add)
                            for tti, t in enumerate(stts):
                                tp_ps = PSM.tile([128, HIDDEN], f16,
                                                 tag="tp")
                                nc.tensor.transpose(
                                    tp_ps[:, :N_CLASSES],
                                    o_sb[:, tti * 128:(tti + 1) * 128],
                                    ident[:N_CLASSES, :N_CLASSES])
                                ot = S.tile([128, N_CLASSES], f32, tag="ot")
                                nc.vector.tensor_copy(
                                    ot[:], tp_ps[:, :N_CLASSES])
                                nc.sync.dma_start(
                                    out_dram[t * 128:(t + 1) * 128, :],
                                    ot[:])
                if not is_last:
                    nc.gpsimd.collective_compute(
                        "AllGather", Alu.bypass, replica_groups=rg,
                        ins=[nxt_shard.opt()],
                        outs=[hs_full[li + 1].opt()])

    nc.compile()
    return nc


def kernel(x, edge_index, W0, b0, Wl, W_out, b_out):
    from concourse.bass_utils import run_bass_kernel_spmd

    x = np.asarray(x, dtype=np.float32)
    edge_index = np.asarray(edge_index)
    W0 = np.asarray(W0, dtype=np.float32)
    b0 = np.asarray(b0, dtype=np.float32)
    Wl = np.asarray(Wl, dtype=np.float32)
    W_out = np.asarray(W_out, dtype=np.float32)
    b_out = np.asarray(b_out, dtype=np.float32)

    struct, per_core = _preprocess(edge_index)
    nc = _build_program(struct)

    betas = np.array(
        [math.log(LAMDA / (i + 1) + 1.0) for i in range(N_LAYERS)],
        dtype=np.float32)
    wl_host = np.zeros((HIDDEN, N_LAYERS * HIDDEN), np.float32)
    eye = np.eye(HIDDEN, dtype=np.float32)
    for i in range(N_LAYERS):
        wl_host[:, i * HIDDEN:(i + 1) * HIDDEN] = (
            betas[i] * Wl[i] + (1.0 - betas[i]) * eye)
    iota_host = np.tile(np.arange(256, dtype=np.float32), (128, 1))
    ident_host = np.eye(128, dtype=np.float32)

    in_maps = []
    for c in range(NCORES):
        pc = per_core[c]
        pm = pc["perm"]
        xp = np.zeros((NPC_PAD, N_FEATS), np.float32)
        valid = pm >= 0
        xp[valid] = x[c * NPC:(c + 1) * NPC][pm[valid]]
        in_maps.append({
            "xt": np.ascontiguousarray(xp.T).astype(np.float16),
            "idx": pc["idx"],
            "tloc": pc["tloc"],
            "dcol": pc["dcol"],
            "dinvo": pc["dinvo"],
            "w0": W0.astype(np.float16),
            "b0": b0.reshape(HIDDEN, 1),
            "wl": wl_host.astype(np.float16),
            "wout": W_out.astype(np.float16),
            "bout": b_out.reshape(N_CLASSES, 1),
            "iota": iota_host.astype(np.float16),
            "ident": ident_host.astype(np.float16),
        })

    res = run_bass_kernel_spmd(
        nc, in_maps, core_ids=list(range(NCORES)), trace=TRACE)
    _LAST_RESULT["res"] = res
    out = np.empty((N_NODES, N_CLASSES), np.float32)
    for c in range(NCORES):
        pm = per_core[c]["perm"]
        valid = pm >= 0
        block = res.results[c]["out"]
        out[c * NPC + pm[valid]] = block[valid]
    return out


# revision 24
# speedup vs baseline: 2.0199x; 1.2180x over previous
"""GCNII (nn_GCNII_17626545783193) Bass/Trainium2 kernel, 8 NeuronCores.

Strategy (target-node sharding, feature-major compute, fp16 data path):
  - Nodes sharded 12500/core (padded to 12544 = 98*128). Edges partitioned
    by target core; self-loops handled via per-tile diagonal matmuls against
    a locally kept node-major hs copy (not in the gather stream).
  - Per-core target permutation balances per-(ttile, chunk) edge counts
    across tiles AND cores, minimizing the 128-slot padding of the uniform
    (cross-core identical) edge stream.
  - gcn_norm factored: norm[e] = dinv[src]*dinv[dst]. dinv[src] is folded
    into the gathered features (hs = dinv * h, fp16 [N, 128]-padded rows,
    256B, replicated via Shared-output AllGather each layer); 0.9*dinv[dst]
    is folded into the one-hot scatter matrix.
  - Per layer, per core: dma_gather of in-edge source rows (256B fp16) from
    hs_full; DVE builds one-hot [128edge, 128target] fp16 (iota==tloc)*dcol;
    PE computes agg^T = msgs^T @ onehot accumulated in PSUM [64, 512] fp32;
    support = agg + 0.1*h0 (fp32 h0, fp16 out); GCNII dense update via ONE
    matmul with host-merged W' = beta*Wl + (1-beta)*I; ACT relu; PE transpose
    to node-major; scale by dinv; strided write into the padded hs rows;
    AllGather.
"""
import sys
sys.path.insert(0, "/opt/trn_rl_repo")

import math
import numpy as np

# ---- problem constants (hardcoded per spec) ----
N_NODES = 100000
N_FEATS = 128
HIDDEN = 64
N_CLASSES = 40
N_LAYERS = 8
ALPHA = 0.1
LAMDA = 0.5
NCORES = 8

NPC = N_NODES // NCORES            # 12500 real nodes per core
TPC = (NPC + 127) // 128           # 98 target tiles per core
NPC_PAD = TPC * 128                # 12544
N_PAD = NPC_PAD * NCORES           # 100352
HROW = 128                         # padded fp16 row elems (256B)
GROUP_TT = 8                       # target tiles per gather group
SUB_TT = 4                         # target tiles per psum tile (512 cols)
SRC_CHUNK = 32768                  # int16 index range per gather source chunk
NCHUNKS = (N_PAD + SRC_CHUNK - 1) // SRC_CHUNK  # 4
CALL_MAX = 8192                    # max indices per dma_gather call

TRACE = False          # test.py sets this for profiling
_LAST_RESULT = {}      # test.py reads exec_time from here


def _balance_tiles(deg_ck):
    """Assign 12500 local targets to 98 tiles of <=128, balancing total
    degree: sort by degree desc, snake-deal. Returns perm: perm[new_pos] =
    old local id, laid out tile-major (tile t = perm[t*128:(t+1)*128],
    padded with -1)."""
    tot = deg_ck.sum(axis=1)
    order = np.argsort(-tot, kind="stable")
    tiles = [[] for _ in range(TPC)]
    tsum = np.zeros(TPC)
    # snake deal in rounds of TPC
    i = 0
    fwd = True
    while i < len(order):
        rng = range(TPC) if fwd else range(TPC - 1, -1, -1)
        for t in rng:
            if i >= len(order):
                break
            if len(tiles[t]) < 128:
                tiles[t].append(order[i])
                tsum[t] += tot[order[i]]
                i += 1
        fwd = not fwd
    # order tiles by total degree desc so maxima align across cores
    tile_order = np.argsort(-tsum, kind="stable")
    perm = np.full(NPC_PAD, -1, np.int64)
    for newt, oldt in enumerate(tile_order):
        ids = tiles[oldt]
        perm[newt * 128:newt * 128 + len(ids)] = ids
    return perm


def _preprocess(edge_index):
    """Build the uniform (cross-core identical) edge stream structure and the
    per-core data arrays (with per-core balancing permutations)."""
    row = np.asarray(edge_index[0], dtype=np.int64)
    col = np.asarray(edge_index[1], dtype=np.int64)
    deg = np.bincount(col, minlength=N_NODES).astype(np.float32) + 1.0
    dinv = (1.0 / np.sqrt(deg)).astype(np.float32)

    # self-loops are handled on-chip via a per-tile diagonal matmul against
    # the locally-kept node-major hs copy — they are NOT in the edge stream.
    r = row
    cl = col
    core = cl // NPC
    lcol = cl - core * NPC                       # 0..12499 local target

    # ---- per-core balancing permutation over local targets ----
    src_core = r // NPC
    src_local = r - src_core * NPC
    perms = []            # perm[new_pos (padded)] = old local id
    inv_perms = []        # inv[old local id] = new_pos (padded)
    new_lcol = np.empty_like(lcol)
    for c in range(NCORES):
        m = core == c
        # per-target degree per source chunk (chunk of the *old* padded gp;
        # balancing on totals is enough, chunks are ~proportional)
        deg_c = np.bincount(lcol[m], minlength=NPC)
        perm = _balance_tiles(deg_c.reshape(-1, 1).astype(np.float64))
        inv = np.full(NPC, -1, np.int64)
        valid = perm >= 0
        inv[perm[valid]] = np.nonzero(valid)[0]
        perms.append(perm)
        inv_perms.append(inv)
        new_lcol[m] = inv[lcol[m]]
    lcol = new_lcol                               # padded-permuted local target

    # padded global source id (through the source core's permutation)
    gp = np.empty_like(r)
    for c in range(NCORES):
        m = src_core == c
        gp[m] = c * NPC_PAD + inv_perms[c][src_local[m]]

    # pair-granular slots: each 128-edge chunk targets one PAIR of ttiles
    # (256 targets) via a [128, 256] one-hot.
    NPAIR = TPC // 2                             # 49
    pr = lcol >> 8                               # target pair 0..48
    k = gp // SRC_CHUNK                          # source chunk 0..3

    ngroups = (TPC + GROUP_TT - 1) // GROUP_TT

    slot = (core * NPAIR + pr) * NCHUNKS + k
    counts = np.bincount(slot, minlength=NCORES * NPAIR * NCHUNKS).reshape(
        NCORES, NPAIR, NCHUNKS
    )
    maxc = counts.max(axis=0)                    # [NPAIR, NCHUNKS]
    slot_chunks = -(-maxc // 128)                # ceil; 0 allowed
    slot_len = slot_chunks * 128

    GROUP_PR = GROUP_TT // 2
    slot_off = np.zeros((NPAIR, NCHUNKS), np.int64)
    off = 0
    chunk_pr = []
    calls_by_group = [[] for _ in range(ngroups)]
    for g in range(ngroups):
        prs = range(g * GROUP_PR, min((g + 1) * GROUP_PR, NPAIR))
        for kk in range(NCHUNKS):
            run_start = off
            for t in prs:
                slot_off[t, kk] = off
                off += slot_len[t, kk]
                chunk_pr += [t] * slot_chunks[t, kk]
            s = run_start
            while s < off:
                n = min(CALL_MAX, off - s)
                calls_by_group[g].append((kk, int(s), int(n)))
                s += n
    e_pad = off
    nchk = e_pad // 128
    chunk_pr = np.asarray(chunk_pr, np.int64)
    # start/stop are per PSUM tile (= subgroup of SUB_TT//2 pairs): exactly
    # one start=True (chronologically first matmul into the bank) and one
    # stop=True (last).
    sub_id = chunk_pr // (SUB_TT // 2)
    first = np.zeros(nchk, bool)
    last = np.zeros(nchk, bool)
    for sg in np.unique(sub_id):
        js = np.nonzero(sub_id == sg)[0]
        first[js[0]] = True
        last[js[-1]] = True

    per_core = []
    for c in range(NCORES):
        m = core == c
        rc = r[m]
        clm = cl[m]
        lc = lcol[m]
        tc_ = pr[m]
        kc = k[m]
        gpc = gp[m]
        skey = tc_ * NCHUNKS + kc
        order = np.argsort(skey, kind="stable")
        sorted_key = skey[order]
        firsts = np.searchsorted(sorted_key, sorted_key, side="left")
        ranks = np.arange(len(sorted_key)) - firsts
        pos = slot_off[tc_[order], kc[order]] + ranks

        idx_stream = np.zeros(e_pad, np.int16)
        tloc_stream = np.zeros(e_pad, np.float32)
        dcol_stream = np.zeros(e_pad, np.float32)
        idx_stream[pos] = (gpc[order] - kc[order] * SRC_CHUNK).astype(np.int16)
        tloc_stream[pos] = (lc[order] & 255).astype(np.float32)
        dcol_stream[pos] = ((1.0 - ALPHA) * dinv[clm[order]]).astype(np.float32)

        idx_w = np.zeros((128, e_pad // 16), np.int16)
        for g in range(ngroups):
            for (kk, s, n) in calls_by_group[g]:
                blk = idx_stream[s:s + n].reshape(n // 16, 16).T
                idx_w[:, s // 16:(s + n) // 16] = np.tile(blk, (8, 1))
        tloc_arr = np.ascontiguousarray(tloc_stream.reshape(nchk, 128).T)
        dcol_arr = np.ascontiguousarray(dcol_stream.reshape(nchk, 128).T)
        dv = np.zeros(NPC_PAD, np.float32)
        pm = perms[c]
        valid = pm >= 0
        dv[valid] = dinv[c * NPC + pm[valid]]
        dinvo = np.ascontiguousarray(dv.reshape(TPC, 128).T)
        per_core.append(
            dict(idx=idx_w, tloc=tloc_arr, dcol=dcol_arr, dinvo=dinvo,
                 perm=pm)
        )

    struct = dict(
        e_pad=e_pad,
        nchk=nchk,
        ngroups=ngroups,
        calls_by_group=calls_by_group,
        chunk_pr=chunk_pr,
        first=first,
        last=last,
    )
    return struct, per_core


def _build_program(struct):
    import concourse.bass as bass
    import concourse.mybir as mybir
    import concourse.tile as tile
    import concourse.bacc as bacc

    dt = mybir.dt
    f32 = dt.float32
    f16 = dt.float16
    Alu = mybir.AluOpType
    Act = mybir.ActivationFunctionType

    e_pad = struct["e_pad"]
    nchk = struct["nchk"]
    ngroups = struct["ngroups"]
    calls_by_group = struct["calls_by_group"]
    chunk_pr = struct["chunk_pr"]
    first = struct["first"]
    last = struct["last"]
    rg = [list(range(NCORES))]

    nc = bacc.Bacc("TRN2", target_bir_lowering=False, debug=False,
                   num_devices=NCORES, num_swdge_queues=4)

    xt_in = nc.dram_tensor("xt", [N_FEATS, NPC_PAD], f16, kind="ExternalInput")
    idx_in = nc.dram_tensor("idx", [128, e_pad // 16], dt.int16,
                            kind="ExternalInput")
    tloc_in = nc.dram_tensor("tloc", [128, nchk], f32, kind="ExternalInput")
    dcol_in = nc.dram_tensor("dcol", [128, nchk], f32, kind="ExternalInput")
    dinvo_in = nc.dram_tensor("dinvo", [128, TPC], f32, kind="ExternalInput")
    w0_in = nc.dram_tensor("w0", [N_FEATS, HIDDEN], f16, kind="ExternalInput")
    b0_in = nc.dram_tensor("b0", [HIDDEN, 1], f32, kind="ExternalInput")
    wl_in = nc.dram_tensor("wl", [HIDDEN, N_LAYERS * HIDDEN], f16,
                           kind="ExternalInput")
    wout_in = nc.dram_tensor("wout", [HIDDEN, N_CLASSES], f16,
                             kind="ExternalInput")
    bout_in = nc.dram_tensor("bout", [N_CLASSES, 1], f32, kind="ExternalInput")
    iota_in = nc.dram_tensor("iota", [128, 256], f16, kind="ExternalInput")
    ident_in = nc.dram_tensor("ident", [128, 128], f16, kind="ExternalInput")
    out_dram = nc.dram_tensor("out", [NPC_PAD, N_CLASSES], f32,
                              kind="ExternalOutput")

    with tile.TileContext(nc) as tc:
        with (
            tc.tile_pool(name="persist", bufs=1) as P,
            tc.tile_pool(name="work", bufs=3) as S,
            tc.tile_pool(name="msgp", bufs=7) as MSG,
            tc.tile_pool(name="ohp", bufs=12) as OH,
            tc.tile_pool(name="dgp", bufs=8) as DG,
            tc.tile_pool(name="psagg", bufs=4, space="PSUM") as PSA,
            tc.tile_pool(name="psmisc", bufs=2, space="PSUM") as PSM,
            tc.tile_pool(name="dram", bufs=1, space="DRAM") as DR,
        ):
            def pload(name, src, shape, dtype=f32):
                t = P.tile(shape, dtype, tag=name)
                nc.sync.dma_start(t[:], src[:])
                return t

            iota = pload("iota", iota_in, [128, 256], f16)
            ident = pload("ident", ident_in, [128, 128], f16)
            w0 = pload("w0", w0_in, [N_FEATS, HIDDEN], f16)
            b0 = pload("b0", b0_in, [HIDDEN, 1])
            wl = pload("wl", wl_in, [HIDDEN, N_LAYERS * HIDDEN], f16)
            wout = pload("wout", wout_in, [HIDDEN, N_CLASSES], f16)
            bout = pload("bout", bout_in, [N_CLASSES, 1])
            tloc = pload("tloc", tloc_in, [128, nchk])
            dcol = pload("dcol", dcol_in, [128, nchk])
            dinvo = pload("dinvo", dinvo_in, [128, TPC])
            # layer-invariant gather indices, resident in SBUF
            idxs = pload("idxs", idx_in, [128, e_pad // 16], dt.int16)
            h0s = P.tile([HIDDEN, NPC_PAD], f32, tag="h0s")
            # node-major hs of the current layer, kept locally for the
            # self-loop diagonal matmul (also the staging buffer for the
            # hs_shard DMA writes)
            hsl = P.tile([128, TPC, HIDDEN], f16, tag="hsl")

            hs_shard = [
                DR.tile([NPC_PAD, HROW], f16, tag=f"shard{j}",
                        name=f"hs_shard{j}")
                for j in range(N_LAYERS)
            ]
            hs_full = [
                DR.tile([N_PAD, HROW], f16, tag=f"full{j}",
                        addr_space="Shared", name=f"hs_full{j}")
                for j in range(N_LAYERS)
            ]

            def emit_node_tile(ht, tti, t, nxt_shard):
                """Transpose ht[:, tti*128:...] (fp16 [64,128]) to node-major,
                scale by dinv into the persistent hsl buffer, write the
                padded hs row."""
                tp_ps = PSM.tile([128, HIDDEN], f16, tag="tp")
                nc.tensor.transpose(
                    tp_ps[:], ht[:, tti * 128:(tti + 1) * 128],
                    ident[:HIDDEN, :HIDDEN])
                nc.vector.tensor_scalar(hsl[:, t, :], tp_ps[:],
                                        dinvo[:, t:t + 1], None,
                                        op0=Alu.mult)
                nc.sync.dma_start(
                    nxt_shard[t * 128:(t + 1) * 128, 0:HIDDEN], hsl[:, t, :])

            # ---------------- layer 0: h0 = relu(x @ W0 + b0) ----------------
            for bi in range((TPC + SUB_TT - 1) // SUB_TT):
                t0 = bi * SUB_TT
                w = min(SUB_TT, TPC - t0) * 128
                xt_sb = S.tile([N_FEATS, SUB_TT * 128], f16, tag="xt")
                nc.sync.dma_start(
                    xt_sb[:, :w], xt_in[:, t0 * 128:t0 * 128 + w])
                h_ps = PSM.tile([HIDDEN, SUB_TT * 128], f32, tag="dense")
                nc.tensor.matmul(h_ps[:, :w], lhsT=w0[:], rhs=xt_sb[:, :w],
                                 start=True, stop=True)
                h0t = S.tile([HIDDEN, SUB_TT * 128], f16, tag="ht")
                nc.scalar.activation(h0t[:, :w], h_ps[:, :w], Act.Relu,
                                     bias=b0[:])
                nc.vector.tensor_scalar_mul(
                    h0s[:, t0 * 128:t0 * 128 + w], h0t[:, :w], ALPHA)
                for tti in range(w // 128):
                    emit_node_tile(h0t, tti, t0 + tti, hs_shard[0])
            nc.gpsimd.collective_compute(
                "AllGather", Alu.bypass, replica_groups=rg,
                ins=[hs_shard[0].opt()], outs=[hs_full[0].opt()])

            # ---------------- GCNII layers ----------------
            qctr = [0]
            nreg = {}
            for g in range(ngroups):
                for (kk, s_, n) in calls_by_group[g]:
                    if n not in nreg:
                        nreg[n] = nc.gpsimd.to_reg(n)
            for li in range(N_LAYERS):
                cur = hs_full[li]
                is_last = li == N_LAYERS - 1
                nxt_shard = hs_shard[li + 1] if not is_last else None
                for g in range(ngroups):
                    tts = list(range(g * GROUP_TT,
                                     min((g + 1) * GROUP_TT, TPC)))
                    subs = [tts[i:i + SUB_TT]
                            for i in range(0, len(tts), SUB_TT)]
                    ps_tiles = [PSA.tile([HIDDEN, SUB_TT * 128], f32,
                                         tag="agg", name=f"agg{li}_{g}_{si}")
                                for si in range(len(subs))]
                    # self-loop contributions: agg^T[:, t] += hs_t^T @ dg_t
                    # (hsl holds this layer's hs, written at end of layer-1;
                    # dg_t = ident * 0.9*dinv_t built on the fly)
                    for si, stts in enumerate(subs):
                        for tti, t in enumerate(stts):
                            dgt = DG.tile([128, 128], f16, tag="dg")
                            nc.vector.tensor_scalar(
                                dgt[:], ident[:], dinvo[:, t:t + 1],
                                1.0 - ALPHA, op0=Alu.mult, op1=Alu.mult)
                            nc.tensor.matmul(
                                ps_tiles[si][:, tti * 128:(tti + 1) * 128],
                                lhsT=hsl[:, t, :], rhs=dgt[:],
                                start=(tti == 0), stop=False,
                                skip_group_check=True)
                    for (kk, s, n) in calls_by_group[g]:
                        msg = MSG.tile([128, n // 128, HROW], f16,
                                       tag="msg")
                        rows_k = min(SRC_CHUNK, N_PAD - kk * SRC_CHUNK)
                        nc.gpsimd.dma_gather(
                            msg[:],
                            cur[kk * SRC_CHUNK:kk * SRC_CHUNK + rows_k, :],
                            idxs[:, s // 16:(s + n) // 16],
                            num_idxs=n, num_idxs_reg=nreg[n],
                            elem_size=HROW, single_packet=False,
                            queue_num=qctr[0] % 4)
                        qctr[0] += 1
                        for jj in range(n // 128):
                            j = s // 128 + jj
                            pj = int(chunk_pr[j])
                            oh = OH.tile([128, 256], f16, tag="oh")
                            nc.vector.tensor_scalar(
                                oh[:], iota[:], tloc[:, j:j + 1],
                                dcol[:, j:j + 1],
                                op0=Alu.is_equal, op1=Alu.mult)
                            si = (pj - g * (GROUP_TT // 2)) // (SUB_TT // 2)
                            ci = (pj % (SUB_TT // 2)) * 256
                            nc.tensor.matmul(
                                ps_tiles[si][:, ci:ci + 256],
                                lhsT=msg[:, jj, 0:HIDDEN], rhs=oh[:],
                                start=False, stop=bool(last[j]),
                                skip_group_check=True)
                    for si, stts in enumerate(subs):
                        w = len(stts) * 128
                        n0 = stts[0] * 128
                        sup = S.tile([HIDDEN, SUB_TT * 128], f16, tag="sup")
                        nc.vector.tensor_tensor(
                            out=sup[:, :w], in0=ps_tiles[si][:, :w],
                            in1=h0s[:, n0:n0 + w], op=Alu.add)
                        d_ps = PSM.tile([HIDDEN, SUB_TT * 128], f32,
                                        tag="dense")
                        nc.tensor.matmul(
                            d_ps[:, :w],
                            lhsT=wl[:, li * HIDDEN:(li + 1) * HIDDEN],
                            rhs=sup[:, :w], start=True, stop=True)
                        if not is_last:
                            ht = S.tile([HIDDEN, SUB_TT * 128], f16,
                                        tag="ht")
                            nc.scalar.activation(ht[:, :w], d_ps[:, :w],
                                                 Act.Relu)
                            for tti, t in enumerate(stts):
                                emit_node_tile(ht, tti, t, nxt_shard)
                        else:
                            ht = S.tile([HIDDEN, SUB_TT * 128], f16,
                                        tag="ht")
                            nc.scalar.activation(ht[:, :w], d_ps[:, :w],
                                                 Act.Relu)
                            o_ps_full = PSM.tile([HIDDEN, SUB_TT * 128], f32,
                                                 tag="dense")
                            o_ps = o_ps_full[:N_CLASSES, :]
                            nc.tensor.matmul(o_ps[:, :w], lhsT=wout[:],
                                             rhs=ht[:, :w],
                                             start=True, stop=True)
                            o_sb = S.tile([N_CLASSES, SUB_TT * 128], f16,
                                          tag="osb")
                            nc.vector.tensor_scalar(
                                o_sb[:, :w], o_ps[:, :w], bout[:], None,
                                op0=Alu.add)
                            for tti, t in enumerate(stts):
                                tp_ps = PSM.tile([128, HIDDEN], f16,
                                                 tag="tp")
                                nc.tensor.transpose(
                                    tp_ps[:, :N_CLASSES],
                                    o_sb[:, tti * 128:(tti + 1) * 128],
                                    ident[:N_CLASSES, :N_CLASSES])
                                ot = S.tile([128, N_CLASSES], f32, tag="ot")
                                nc.vector.tensor_copy(
                                    ot[:], tp_ps[:, :N_CLASSES])
                                nc.sync.dma_start(
                                    out_dram[t * 128:(t + 1) * 128, :],
                                    ot[:])
                if not is_last:
                    nc.gpsimd.collective_compute(
                        "AllGather", Alu.bypass, replica_groups=rg,
                        ins=[nxt_shard.opt()],
                        outs=[hs_full[li + 1].opt()])

    nc.compile()
    return nc


def kernel(x, edge_index, W0, b0, Wl, W_out, b_out):
    from concourse.bass_utils import run_bass_kernel_spmd

    x = np.asarray(x, dtype=np.float32)
    edge_index = np.asarray(edge_index)
    W0 = np.asarray(W0, dtype=np.float32)
    b0 = np.asarray(b0, dtype=np.float32)
    Wl = np.asarray(Wl, dtype=np.float32)
    W_out = np.asarray(W_out, dtype=np.float32)
    b_out = np.asarray(b_out, dtype=np.float32)

    struct, per_core = _preprocess(edge_index)
    nc = _build_program(struct)

    betas = np.array(
        [math.log(LAMDA / (i + 1) + 1.0) for i in range(N_LAYERS)],
        dtype=np.float32)
    wl_host = np.zeros((HIDDEN, N_LAYERS * HIDDEN), np.float32)
    eye = np.eye(HIDDEN, dtype=np.float32)
    for i in range(N_LAYERS):
        wl_host[:, i * HIDDEN:(i + 1) * HIDDEN] = (
            betas[i] * Wl[i] + (1.0 - betas[i]) * eye)
    iota_host = np.tile(np.arange(256, dtype=np.float32), (128, 1))
    ident_host = np.eye(128, dtype=np.float32)

    in_maps = []
    for c in range(NCORES):
        pc = per_core[c]
        pm = pc["perm"]
        xp = np.zeros((NPC_PAD, N_FEATS), np.float32)
        valid = pm >= 0
        xp[valid] = x[c * NPC:(c + 1) * NPC][pm[valid]]
        in_maps.append({
            "xt": np.ascontiguousarray(xp.T).astype(np.float16),
            "idx": pc["idx"],
            "tloc": pc["tloc"],
            "dcol": pc["dcol"],
            "dinvo": pc["dinvo"],
            "w0": W0.astype(np.float16),
            "b0": b0.reshape(HIDDEN, 1),
            "wl": wl_host.astype(np.float16),
            "wout": W_out.astype(np.float16),
            "bout": b_out.reshape(N_CLASSES, 1),
            "iota": iota_host.astype(np.float16),
            "ident": ident_host.astype(np.float16),
        })

    res = run_bass_kernel_spmd(
        nc, in_maps, core_ids=list(range(NCORES)), trace=TRACE)
    _LAST_RESULT["res"] = res
    out = np.empty((N_NODES, N_CLASSES), np.float32)
    for c in range(NCORES):
        pm = per_core[c]["perm"]
        valid = pm >= 0
        block = res.results[c]["out"]
        out[c * NPC + pm[valid]] = block[valid]
    return out
